# revision 1
# baseline (speedup 1.0000x reference)
"""Trainium2 Bass kernel for nn_MiniDSARouter (topk block routing).

Problem (hardcoded shapes): B=2, T=8192, HQ=32, H=8, D=64, DR=16,
block_size=64, selected_blocks=16, groups=4, ADD_LOCAL=1.

Reference semantics (verified equivalent):
  out[b,t,h,:] = sorted(top16_idx(scores[b,t,h,:]) ++ [t_blk, max(t_blk-1,0)])[:16]
where scores = (Qrep @ Wq) @ (blockmean(K) @ Wk)^T with causal block mask
(blocks > t//64 masked to -inf), and the positive per-head scale
exp(logit_scale) never changes the ranking so it is dropped.
The sequential "dedup" in the reference is numerically a no-op.
For t_blk <= 15 the top-16 set is always {0..15} (mask fill), so those rows
are a static function of t and are written from a precomputed table.

Sharding: one KV head per NeuronCore (8 heads / 8 cores). All work is
per-(b,t,h) so there is no cross-core communication.
"""

import numpy as np

import concourse.bass as bass
import concourse.mybir as mybir
import concourse.tile as tile
from concourse import bacc
from concourse.bass_utils import run_bass_kernel_spmd

B, T, HQ, H, D, DR = 2, 8192, 32, 8, 64, 16
BS = 64                    # block size
NB = T // BS               # 128 blocks per batch
SEL = 16                   # selected blocks
GROUPS = 4
ROWS = B * T               # 16384 rows per core (all t of both batches, one head)
NTILES_SKIP = 8            # per-batch tiles 0..7 (t < 1024) are static
TPB = T // 128             # 64 row-tiles of 128 per batch
NGT = TPB - NTILES_SKIP    # 56 computed tiles per batch
NG = B * NGT               # 112 computed tiles per core
NEG_BIG = -1e30

_CACHE = {}


def _tiles():
    """Computed tiles: g -> (b, i). b0 ascending, b1 descending so the
    pipeline drains on the cheapest (smallest causal width) tiles."""
    return ([(0, NTILES_SKIP + k) for k in range(NGT)]
            + [(1, TPB - 1 - k) for k in range(NGT)])


def _gcol(g):
    """Output-group column for loop position g (ascending i within group)."""
    r = g % GB_
    return r if g < NGT else GB_ - 1 - r


GB_ = 14


def _static_tables():
    # early rows: t_blk <= 15 -> sorted([0..15] + [t_blk, max(t_blk-1,0)])[:16]
    # laid out [p, j, :] = row t = j*128 + p  (SBUF partition tiling)
    early = np.empty((128, NTILES_SKIP, SEL), np.int32)
    for t in range(NTILES_SKIP * 128):
        tb = t // BS
        s = sorted(list(range(16)) + [tb, max(tb - 1, 0)])
        early[t % 128, t // 128] = s[:SEL]
    early = early.reshape(128, NTILES_SKIP * SEL)
    # loc1[p, g] = t_blk - 1 for the row at (partition p, computed tile g):
    # final clamp out[15] = min(out[15], t_blk - 1)
    loc1 = np.empty((128, NG), np.uint32)
    for g, (b, i) in enumerate(_tiles()):
        col = (g // GB_) * GB_ + _gcol(g)
        for p in range(128):
            loc1[p, col] = 2 * i + (1 if p >= 64 else 0) - 1
    # descending constants for max_index on the prefix-scan: [-1..-16]
    cdesc = np.tile(-np.arange(1, 17, dtype=np.float32), (128, 1))
    # zap column: add -1e30 to (p < 64) rows of the tile's odd column
    zap = np.zeros((1, 128), np.float32)
    zap[0, :64] = NEG_BIG
    one = np.ones((1, 1), np.float32)
    return early, loc1, cdesc, zap, one


def build_program():
    f32 = mybir.dt.float32
    nc = bacc.Bacc("TRN2", target_bir_lowering=False, debug=False)

    qT_d = nc.dram_tensor("qT", [D, ROWS], f32, kind="ExternalInput")
    kT_d = nc.dram_tensor("kT", [2 * D, ROWS // 2], f32, kind="ExternalInput")
    wq_d = nc.dram_tensor("wq", [D, DR], f32, kind="ExternalInput")
    wks_d = nc.dram_tensor("wks", [2 * D, DR], f32, kind="ExternalInput")
    loc1_d = nc.dram_tensor("loc1", [128, NG], mybir.dt.uint32, kind="ExternalInput")
    cdesc_d = nc.dram_tensor("cdesc", [128, SEL], f32, kind="ExternalInput")
    zap_d = nc.dram_tensor("zap", [1, 128], f32, kind="ExternalInput")
    one_d = nc.dram_tensor("one", [1, 1], f32, kind="ExternalInput")
    early_d = nc.dram_tensor("early", [128, NTILES_SKIP * SEL], mybir.dt.uint32,
                             kind="ExternalInput")
    out_d = nc.dram_tensor("out", [ROWS, SEL], mybir.dt.uint32,
                           kind="ExternalOutput")

    with tile.TileContext(nc) as tc:
        with (
            tc.tile_pool(name="singles", bufs=1) as singles,
            tc.tile_pool(name="kchunk", bufs=2) as kpool,
            tc.tile_pool(name="qchunk", bufs=6) as qpool,
            tc.tile_pool(name="tree", bufs=1) as tree,
            tc.tile_pool(name="qr_ps", bufs=2, space="PSUM") as qr_ps,
            tc.tile_pool(name="sc_ps", bufs=6, space="PSUM") as sc_ps,
            tc.tile_pool(name="sc_sb", bufs=20) as scpool,
            tc.tile_pool(name="small", bufs=16) as small,
            tc.tile_pool(name="ogrp", bufs=2) as ogpool,
        ):
            GB = 14
            # ---- static tables / params ----
            wq_sb = singles.tile([D, DR], f32)
            nc.sync.dma_start(out=wq_sb, in_=wq_d.ap())
            wks_sb = singles.tile([2 * D, DR], f32)
            nc.sync.dma_start(out=wks_sb, in_=wks_d.ap())
            zap_sb = singles.tile([1, 128], f32)
            nc.sync.dma_start(out=zap_sb, in_=zap_d.ap())
            one_sb = singles.tile([1, 1], f32)
            nc.sync.dma_start(out=one_sb, in_=one_d.ap())
            out_v = out_d.ap().rearrange("(j p) s -> p j s", p=128)

            # ---- block means: packed 128-partition layout.
            # kT pack c (c=0,1): rows 0:64 = K^T cols of b0 blocks [64c,64c+64),
            # rows 64:128 = K^T cols of b1 blocks [64c,64c+64).
            # ksumT2 [128, 128]: pack c -> cols [64c, 64c+64).
            ksumT2 = singles.tile([2 * D, NB], f32)
            krT_sb = singles.tile([DR, B * NB], f32)
            CH = ROWS // 4                 # 4096 t-cols per pack
            BLK_CH = CH // BS              # 64 blocks per pack half

            def ksum_steps(c, eng):
                """DMA pack c now; return per-level thunks + kr finishers."""
                kc = kpool.tile([2 * D, BLK_CH, BS], f32)
                qtr = CH // 4
                bq = BLK_CH // 4
                for q in range(2):
                    nc.sync.dma_start(
                        out=kc[:, q * bq:(q + 1) * bq, :],
                        in_=kT_d.ap()[:, c * CH + q * qtr:c * CH + (q + 1) * qtr])

                def load_rest():
                    for q in range(2, 4):
                        nc.sync.dma_start(
                            out=kc[:, q * bq:(q + 1) * bq, :],
                            in_=kT_d.ap()[:, c * CH + q * qtr:
                                          c * CH + (q + 1) * qtr])
                steps = []
                state = {"cur": kc, "w": BS}

                def level_part(nsub, sub):
                    def f():
                        w = state["w"] // 2
                        cur = state["cur"]
                        blo = BLK_CH * sub // nsub
                        bhi = BLK_CH * (sub + 1) // nsub
                        if w == 1:
                            dst = ksumT2[:, c * BLK_CH + blo:c * BLK_CH + bhi]
                            eng.tensor_add(dst, cur[:, blo:bhi, 0:1],
                                           cur[:, blo:bhi, 1:2])
                        else:
                            if sub == 0:
                                nxt_tile = tree.tile([2 * D, BLK_CH, w], f32,
                                                     name=f"tr{c}_{w}",
                                                     tag=f"tree{c}_{w}")
                                state["nxt"] = nxt_tile
                            nxt = state["nxt"]
                            eng.tensor_add(nxt[:, blo:bhi, :],
                                           cur[:, blo:bhi, 0:w],
                                           cur[:, blo:bhi, w:2 * w])
                        if sub == nsub - 1:
                            if w > 1:
                                state["cur"] = state["nxt"]
                            state["w"] = w
                    return f

                def finish():
                    # pack c rows 0:64 -> b0 blocks [64c,64c+64);
                    # rows 64:128 -> b1 blocks [64c,64c+64)
                    for bb in range(B):
                        kr_psum = sc_ps.tile([DR, BLK_CH], f32,
                                             name=f"krps{c}_{bb}", tag="scps")
                        nc.tensor.matmul(
                            kr_psum, lhsT=wks_sb[bb * D:(bb + 1) * D, :],
                            rhs=ksumT2[bb * D:(bb + 1) * D,
                                       c * BLK_CH:(c + 1) * BLK_CH],
                            start=True, stop=True)
                        nc.scalar.copy(
                            out=krT_sb[:, bb * NB + c * BLK_CH:
                                       bb * NB + (c + 1) * BLK_CH],
                            in_=kr_psum)
                steps.append(load_rest)
                for w, nsub in ((32, 4), (16, 2), (8, 1), (4, 1), (2, 1), (1, 1)):
                    for sub in range(nsub):
                        steps.append(level_part(nsub, sub))
                steps.append(finish)
                return steps

            def ksum_chunk(c, eng):
                for s in ksum_steps(c, eng):
                    s()

            st0 = ksum_steps(0, nc.vector)
            # ---- qrT[r, row] = Wq^T @ qT, computed lazily per 512-col chunk
            # (interleaved with the score tiles; chunks covering only
            #  never-scored early rows are skipped entirely)
            qrT_sb = singles.tile([DR, ROWS], f32)
            QC = 512
            qr_done = set()
            qr_loaded = {}

            def ensure_qload(col_lo, col_hi):
                col_lo = max(col_lo, 0)
                col_hi = min(col_hi, ROWS)
                for c in range(col_lo // QC, (col_hi + QC - 1) // QC):
                    if c in qr_done or c in qr_loaded:
                        continue
                    qc = qpool.tile([D, QC], f32, name=f"qc{c}", tag="qc")
                    nc.sync.dma_start(out=qc,
                                      in_=qT_d.ap()[:, c * QC:(c + 1) * QC])
                    qr_loaded[c] = qc

            def ensure_qr(col_lo, col_hi):
                for c in range(col_lo // QC, (col_hi + QC - 1) // QC):
                    if c in qr_done:
                        continue
                    if c not in qr_loaded:
                        ensure_qload(c * QC, (c + 1) * QC)
                    qr_done.add(c)
                    qc = qr_loaded.pop(c)
                    ps = qr_ps.tile([DR, QC], f32, tag="qrps")
                    nc.tensor.matmul(ps, lhsT=wq_sb, rhs=qc,
                                     start=True, stop=True)
                    nc.scalar.copy(out=qrT_sb[:, c * QC:(c + 1) * QC], in_=ps)

            # ---- per row-tile: scores, top-16, merge locals, sorted out ----
            # prefetch qr for the first few tiles before the remaining kT loads
            ensure_qr(NTILES_SKIP * 128, NTILES_SKIP * 128 + 2 * QC)
            for s in st0:
                s()

            # non-critical tables + static early rows (queued after the
            # critical kT pack 0 / qT loads)
            loc1_sb = singles.tile([128, NG], mybir.dt.uint32)
            nc.sync.dma_start(out=loc1_sb, in_=loc1_d.ap())
            cdesc_sb = singles.tile([128, SEL], f32)
            nc.sync.dma_start(out=cdesc_sb, in_=cdesc_d.ap())
            early_sb = singles.tile([128, NTILES_SKIP, SEL], mybir.dt.uint32)
            nc.sync.dma_start(out=early_sb, in_=early_d.ap())
            for b in range(B):
                jb = b * TPB
                nc.sync.dma_start(out=out_v[:, jb:jb + NTILES_SKIP, :],
                                  in_=early_sb)

            # kr chunk c is needed from: c0 -> (b0, i<32), c1 -> (b0, i>=32),
            # c2 -> (b1, i<32), c3 -> (b1, i>=32). Chunk work is spread one
            # op per tile iteration so Pool's in-order stream never stalls.
            enqueue_at = {4: (1, nc.vector)}
            deadline = {24: 1}
            pending = []

            ogrp = None
            def emit_tile(g, sc):
                b, i = _tiles()[g]
                W = 2 * i + 2
                nonlocal ogrp
                v = small.tile([128, 16], f32, name=f"v{g}", tag="v")
                sc2 = scpool.tile([128, 128], f32, name=f"sc2_{g}", tag="sc2")
                nc.vector.max(out=v[:, 0:8], in_=sc[:, :W])
                nc.vector.match_replace(out=sc2[:, :W], in_to_replace=v[:, 0:8],
                                        in_values=sc[:, :W], imm_value=NEG_BIG)
                nc.vector.max(out=v[:, 8:16], in_=sc2[:, :W])

                # cneg[j] = -1 if sc[j] >= tau (16th largest) else 0
                cneg = scpool.tile([128, 128], f32, name=f"cneg{g}", tag="cneg")
                nc.gpsimd.tensor_scalar(cneg[:, :W], sc[:, :W], v[:, 15:16],
                                        -1.0, op0=mybir.AluOpType.is_ge,
                                        op1=mybir.AluOpType.mult)
                # P = prefix sum of cneg: hits -1..-16 at the sorted positions
                P = scpool.tile([128, 128], f32, name=f"P{g}", tag="pscan")
                nc.vector.tensor_tensor_scan(P[:, :W], cneg[:, :W], cneg[:, :W],
                                             0.0, op0=mybir.AluOpType.add,
                                             op1=mybir.AluOpType.bypass)

                gi = _gcol(g)
                if g % GB == 0:
                    ogrp = ogpool.tile([128, GB, SEL], mybir.dt.uint32,
                                       name=f"og{g}", tag="ogrp")
                o2a = ogrp[:, gi:gi + 1, 0:8].rearrange("p a b -> p (a b)")
                o2b = ogrp[:, gi:gi + 1, 8:16].rearrange("p a b -> p (a b)")
                nc.vector.max_index(out=o2a, in_max=cdesc_sb[:, 0:8],
                                    in_values=P[:, :W])
                nc.vector.max_index(out=o2b, in_max=cdesc_sb[:, 8:16],
                                    in_values=P[:, :W])
                if g % GB == GB - 1:
                    # clamp each tile's last slot with t_blk-1, then store
                    lastcol = ogrp[:, :, 15:16].rearrange("p a b -> p (a b)")
                    nc.vector.tensor_tensor(lastcol, lastcol,
                                            loc1_sb[:, g - GB + 1:g + 1],
                                            mybir.AluOpType.min)
                    jb = b * TPB + (i - GB + 1 if g < NGT else i)
                    if g == NG - 1:
                        hgb = GB // 2
                        nc.sync.dma_start(out=out_v[:, jb:jb + hgb, :],
                                          in_=ogrp[:, :hgb, :])
                        nc.sync.dma_start(out=out_v[:, jb + hgb:jb + GB, :],
                                          in_=ogrp[:, hgb:, :])
                    else:
                        nc.sync.dma_start(out=out_v[:, jb:jb + GB, :], in_=ogrp)

            ps = None
            sc = None
            for g, (b, i) in enumerate(_tiles()):
                W = 2 * i + 2
                colbase = b * T + i * 128
                if g in enqueue_at:
                    cid, eng = enqueue_at[g]
                    pending.extend(ksum_steps(cid, eng))
                if g in deadline:
                    for s in pending:
                        s()
                    pending.clear()
                elif pending:
                    pending.pop(0)()
                if g < NGT:
                    ensure_qload(colbase + 128, colbase + 128 + 2 * QC)
                else:
                    ensure_qload(colbase - 2 * QC, colbase)
                ensure_qr(colbase, colbase + 128)
                ps = sc_ps.tile([128, 128], f32, name=f"ps{g}", tag="scps")
                nc.tensor.matmul(ps[:, :W],
                                 lhsT=qrT_sb[:, colbase:colbase + 128],
                                 rhs=krT_sb[:, b * NB:b * NB + W],
                                 start=True, stop=False)
                # rows p<64 of this tile must not see block 2i+1
                nc.tensor.matmul(ps[:, 2 * i + 1:2 * i + 2],
                                 lhsT=zap_sb, rhs=one_sb,
                                 start=False, stop=True)

                sc = scpool.tile([128, 128], f32, name=f"sc{g}", tag="sc")
                nc.scalar.copy(out=sc[:, :W], in_=ps[:, :W])
                emit_tile(g, sc)

    nc.compile()
    return nc


def _shard_inputs(Q, K, Wq, Wk):
    early, loc1, cdesc, zap, one = _static_tables()
    early = early.astype(np.uint32)
    in_maps = []
    for h in range(H):
        qT = np.ascontiguousarray(
            Q[:, :, GROUPS * h, :].reshape(ROWS, D).T)
        kTf = K[:, :, h, :].reshape(ROWS, D).T          # [64, 16384]
        half = ROWS // 4
        kT = np.ascontiguousarray(np.hstack([
            np.vstack([kTf[:, 0:half], kTf[:, 2 * half:3 * half]]),
            np.vstack([kTf[:, half:2 * half], kTf[:, 3 * half:4 * half]]),
        ]))                                              # [128, 8192] packed
        in_maps.append({
            "qT": qT.astype(np.float32),
            "kT": kT.astype(np.float32),
            "wq": np.ascontiguousarray(Wq[h]).astype(np.float32),
            "wks": np.ascontiguousarray(
                np.vstack([Wk[h] / 64.0] * 2)).astype(np.float32),
            "loc1": loc1, "cdesc": cdesc, "zap": zap, "one": one,
            "early": early,
        })
    return in_maps


def kernel(Q, K, Wq, Wk, logit_scale=None, block_size=64, selected_blocks=16,
           groups=4, **_unused):
    assert int(block_size) == BS and int(selected_blocks) == SEL
    assert int(groups) == GROUPS
    Q = np.asarray(Q, np.float32)
    K = np.asarray(K, np.float32)
    Wq = np.asarray(Wq, np.float32)
    Wk = np.asarray(Wk, np.float32)
    # exp(logit_scale) > 0 scales scores per-head only -> ranking unchanged.

    if "nc" not in _CACHE:
        _CACHE["nc"] = build_program()
    nc = _CACHE["nc"]

    in_maps = _shard_inputs(Q, K, Wq, Wk)
    res = run_bass_kernel_spmd(nc, in_maps, core_ids=list(range(H)))
    outs = [res.results[h]["out"] for h in range(H)]          # [ROWS, SEL] i32
    out = np.stack(outs, axis=1).reshape(B, T, H, SEL)
    return out.astype(np.int32)


if __name__ == "__main__":
    rng = np.random.default_rng(0)
    Q = rng.standard_normal((B, T, HQ, D)).astype(np.float32)
    K = rng.standard_normal((B, T, H, D)).astype(np.float32)
    Wq = (rng.standard_normal((H, D, DR)) * 0.02).astype(np.float32)
    Wk = (rng.standard_normal((H, D, DR)) * 0.02).astype(np.float32)
    out = kernel(Q=Q, K=K, Wq=Wq, Wk=Wk)
    print("kernel ran:", out.shape, out.dtype)



# revision 2
# speedup vs baseline: 1.0661x; 1.0661x over previous
"""Trainium2 Bass kernel v2 for nn_MiniDSARouter (topk block routing).

Shapes: B=2, T=8192, HQ=32, H=8, D=64, DR=16, block_size=64,
selected_blocks=16, groups=4, ADD_LOCAL=1. One KV head per core.

Semantics (same reduction as baseline, verified vs reference):
  out[b,t,h,:] = sorted_asc(top16_idx(scores[b,t,h,:]))
  with out[15] := min(out[15], t_blk-1)
where scores = q^T (Wq Wk^T/64) ksum^T with causal block mask, and
rows with t_blk <= 15 are a static function of t.

v2 pipeline per 128-row tile (vs baseline's 6 DVE passes + Act copy):
  PE:   scores = qT^T @ M in ONE fp16 matmul (M = G @ blocksum(K),
        G = Wq Wk^T/64 precomputed on host), plus fp16 "zap" matmul
        masking block 2i+1 for rows p<64.
  Act:  copy PSUM fp32 -> SBUF fp16 group buffer.
  DVE:  Max8 / MatchReplace / Max8 -> top-16 values, tau = v[15].
Then per group of 14 tiles (batched ops):
  cpos = (sc >= tau) via one TT is_ge (Pool), with a planted column per
  tile carrying 64*gl; one gated segmented scan (Pool) turns marks into
  per-column rank P + 64*gl; one scalar_tensor_tensor (DVE, 4x fp16)
  maps marked columns to unique bins P+64*gl and unmarked to negative;
  one batched gpsimd local_scatter writes column index j into bin
  rank-1 -- dst bins [64gl+1 .. 64gl+16] ARE the sorted top-16 indices.
  One TT min clamps slot 15 with t_blk-1. DMA out as int16.

All DRAM I/O is fp16/int16 (half the baseline's DMA traffic).
"""

import numpy as np

import concourse.bass as bass
import concourse.mybir as mybir
import concourse.tile as tile
from concourse import bacc
from concourse.bass_utils import run_bass_kernel_spmd

B, T, HQ, H, D, DR = 2, 8192, 32, 8, 64, 16
BS = 64
NB = T // BS               # 128 blocks per batch
SEL = 16
GROUPS = 4
ROWS = B * T               # 16384 rows per core
NTILES_SKIP = 8            # per-batch tiles 0..7 (t < 1024) are static
TPB = T // 128             # 64 row-tiles per batch
NGT = TPB - NTILES_SKIP    # 56 computed tiles per batch
NG = B * NGT               # 112 computed tiles per core
GB = 14                    # tiles per group
NGRP = NG // GB            # 8 groups
BINS = 64                  # scatter bins per tile
BIGC = 1024.0              # unmarked-to-negative shift
ZAPV = -60000.0

_CACHE = {}


def _tiles():
    """b0 ascending i, then b1 descending i (drain on cheap tiles)."""
    return ([(0, NTILES_SKIP + k) for k in range(NGT)]
            + [(1, TPB - 1 - k) for k in range(NGT)])


TILES = _tiles()
# groups: 7x14 tiles + 2x7 (short tail groups drain the pipeline fast)
GRPS = [(14 * k, 14) for k in range(7)] + [(98, 7), (105, 7)]
GRP_WMAX = [max(2 * i + 2 for _, i in TILES[s:s + n]) for s, n in GRPS]
CLASSES = sorted(set(GRP_WMAX))                      # [44, 72, 100, 128]


def _static_tables():
    # early rows: t_blk <= 15 -> sorted([0..15] + [t_blk, max(t_blk-1,0)])[:16]
    early = np.empty((128, NTILES_SKIP, SEL), np.int16)
    for t in range(NTILES_SKIP * 128):
        tb = t // BS
        s = sorted(list(range(16)) + [tb, max(tb - 1, 0)])
        early[t % 128, t // 128] = s[:SEL]
    early = early.reshape(128, NTILES_SKIP * SEL)
    # loc1[p, col] = t_blk - 1, columns in per-group output order
    # (ascending i within each group's output block)
    loc1 = np.empty((128, NG), np.int16)
    col = 0
    for s, n in GRPS:
        iasc = sorted(i for _, i in TILES[s:s + n])
        for i in iasc:
            for p in range(128):
                loc1[p, col] = 2 * i + (1 if p >= 64 else 0) - 1
            col += 1
    # plants: per-tile scan seed = 64*outpos - BIGC, so the stt's
    # (cpos*BIGC + P') lands marked cols at bin 64*outpos + rank and
    # unmarked cols strictly negative. fwd for b0 groups, rev for b1.
    pf = np.tile((np.arange(GB) * BINS - BIGC).astype(np.float16), (128, 1))
    pr = pf[:, ::-1].copy()
    zap = np.zeros((1, 128), np.float16)
    zap[0, :64] = ZAPV
    one = np.ones((1, 1), np.float16)
    blkind = np.zeros((128, 2), np.float16)
    blkind[:64, 0] = 1.0
    blkind[64:, 1] = 1.0
    return early, loc1, pf, pr, zap, one, blkind


def build_program():
    f32 = mybir.dt.float32
    f16 = mybir.dt.float16
    i16 = mybir.dt.int16
    nc = bacc.Bacc("TRN2", target_bir_lowering=False, debug=False)

    qT_d = nc.dram_tensor("qT", [D, ROWS], f16, kind="ExternalInput")
    kc_d = [nc.dram_tensor(f"kc{b}", [128, 64 * D], f16, kind="ExternalInput")
            for b in range(B)]
    gT_d = nc.dram_tensor("gT", [D, D], f32, kind="ExternalInput")
    bun16_d = nc.dram_tensor("bun16", [128, 2 + 2 * GB], f16,
                             kind="ExternalInput")
    zapone_d = nc.dram_tensor("zapone", [1, 129], f16, kind="ExternalInput")
    buni_d = nc.dram_tensor("buni", [128, NG + NTILES_SKIP * SEL], i16,
                            kind="ExternalInput")
    out_d = nc.dram_tensor("out", [ROWS, SEL], i16, kind="ExternalOutput")

    with tile.TileContext(nc) as tc:
        with (
            tc.tile_pool(name="singles", bufs=1) as singles,
            tc.tile_pool(name="qchunk", bufs=1) as qpool,
            tc.tile_pool(name="sc_ps", bufs=8, space="PSUM") as sc_ps,
            tc.tile_pool(name="scg", bufs=4) as scgp,
            tc.tile_pool(name="small", bufs=8) as small,
            tc.tile_pool(name="vpool", bufs=3) as vpool,
            tc.tile_pool(name="taupool", bufs=3) as taupool,
            tc.tile_pool(name="cpool", bufs=2) as cpool,
            tc.tile_pool(name="ppool", bufs=2) as ppool,
            tc.tile_pool(name="upool", bufs=2) as upool,
            tc.tile_pool(name="ixpool", bufs=2) as ixpool,
            tc.tile_pool(name="dstp", bufs=3) as dstp,
        ):
            # ---------------- static tables / params ----------------
            out_v = out_d.ap().rearrange("(j p) s -> p j s", p=128)
            warm = singles.tile([1, 2], f16)
            nc.vector.memset(warm, 0.0)
            warm2 = singles.tile([1, 2], f16)
            nc.scalar.copy(out=warm2, in_=warm)

            # K chunks + blocksum matmuls + M per batch
            kc_sb = [singles.tile([128, 64 * D], f16, name=f"kcs{b}")
                     for b in range(B)]
            M_sb = [singles.tile([D, NB], f16, name=f"Msb{b}")
                    for b in range(B)]
            qT_sb = singles.tile([D, ROWS], f16)
            KQ = 16 * D   # 16 chunks per DMA piece -> 4 pieces per batch

            def load_kc(b, pieces=1):
                n = (64 * D) // pieces
                for q in range(pieces):
                    nc.sync.dma_start(out=kc_sb[b][:, q * n:(q + 1) * n],
                                      in_=kc_d[b].ap()[:, q * n:(q + 1) * n])

            def make_m(b, half=None):
                halves = (0, 1) if half is None else (half,)
                for hf in halves:
                    kp = sc_ps.tile([D, NB // 2], f32, name=f"kps{b}_{hf}",
                                    tag="scps")
                    for c in range(32 * hf, 32 * hf + 32):
                        nc.tensor.matmul(kp[:, 2 * c - 64 * hf:
                                            2 * c + 2 - 64 * hf],
                                         lhsT=kc_sb[b][:, c * D:(c + 1) * D],
                                         rhs=blkind_sb,
                                         start=True, stop=True)
                    ks = small.tile([D, NB // 2], f32, name=f"ksum{b}_{hf}",
                                    tag="ksum")
                    nc.scalar.copy(out=ks, in_=kp)
                    mp = sc_ps.tile([D, NB // 2], f32, name=f"mps{b}_{hf}",
                                    tag="scps")
                    nc.tensor.matmul(mp, lhsT=gT_sb, rhs=ks,
                                     start=True, stop=True)
                    nc.scalar.copy(out=M_sb[b][:, 64 * hf:64 * hf + 64],
                                   in_=mp)

            # critical-path DMA order: kc0, first q chunk, small bundles,
            # kc1, rest of q, index tables
            load_kc(0, pieces=2)
            bun16_sb = singles.tile([128, 2 + 2 * GB], f16)
            nc.sync.dma_start(out=bun16_sb, in_=bun16_d.ap())
            nc.sync.dma_start(out=qT_sb[:, 1024:3072],
                              in_=qT_d.ap()[:, 1024:3072])
            gT_sb = singles.tile([D, D], f32)
            nc.sync.dma_start(out=gT_sb, in_=gT_d.ap())
            zapone_sb = singles.tile([1, 129], f16)
            nc.sync.dma_start(out=zapone_sb, in_=zapone_d.ap())
            blkind_sb = bun16_sb[:, 0:2]
            pf_sb = bun16_sb[:, 2:2 + GB]
            pr_sb = bun16_sb[:, 2 + GB:2 + 2 * GB]
            zap_sb = zapone_sb[:, 0:128]
            one_sb = zapone_sb[:, 128:129]
            make_m(0)
            load_kc(1)
            qranges = [(3072, 5120), (5120, 8192), (12288, 16384),
                       (8192, 12288)]
            for lo, hi in qranges:
                nc.sync.dma_start(out=qT_sb[:, lo:hi],
                                  in_=qT_d.ap()[:, lo:hi])

            buni_sb = singles.tile([128, NG + NTILES_SKIP * SEL], i16)
            nc.sync.dma_start(out=buni_sb, in_=buni_d.ap())
            loc1_sb = buni_sb[:, 0:NG]
            early_v = buni_sb[:, NG:].rearrange(
                "p (a b) -> p a b", a=NTILES_SKIP)
            for b in range(B):
                jb = b * TPB
                nc.sync.dma_start(out=out_v[:, jb:jb + NTILES_SKIP, :],
                                  in_=early_v)

            # gate + iota masters per width class (run during the DMA wait)
            gate_cls = {}
            iota_cls = {}
            for Wc in CLASSES:
                gt = singles.tile([128, GB, Wc + 1], f16, name=f"gate{Wc}")
                nc.gpsimd.memset(gt.rearrange("p a b -> p (a b)"), 1.0)
                nc.gpsimd.memset(gt[:, :, 0:1], 0.0)
                gate_cls[Wc] = gt
                it = singles.tile([128, GB * Wc], i16, name=f"iota{Wc}")
                nc.gpsimd.iota(it[:, :], pattern=[[0, GB], [1, Wc]],
                               base=0, channel_multiplier=0)
                iota_cls[Wc] = it

            # ---------------- main loop (software-pipelined) ----------------
            GRP_COL0 = [sum(n for _, n in GRPS[:G]) for G in range(len(GRPS))]

            def extraction_steps(G, scg, vgrp):
                """Deferred per-group extraction, emitted during group G+1's
                tile loop so the in-order DVE/Pool queues never head-of-line
                block on cross-engine dependencies."""
                s0, gb = GRPS[G]
                tiles = TILES[s0:s0 + gb]
                b = tiles[0][0]
                rev = tiles[0][1] > tiles[-1][1]
                Wc = GRP_WMAX[G]
                # plant slice: fwd groups use pf[:gb]; rev use pr tail
                plant = (pr_sb[:, GB - gb:GB] if rev else pf_sb[:, 0:gb])
                state = {}

                def s_tau():
                    tau32 = taupool.tile([128, gb, 1], mybir.dt.float32,
                                      name=f"tau{G}", tag="tau32")
                    nc.vector.tensor_scalar(tau32, vgrp[:, :, 15:16], 1.0,
                                            0.0, op0=mybir.AluOpType.mult,
                                            op1=mybir.AluOpType.add)
                    cpos = cpool.tile([128, gb, Wc + 1], f16, name=f"cp{G}",
                                     tag="cpos")
                    nc.vector.tensor_scalar(
                        cpos[:, :, 0:1], plant.unsqueeze(2), 1.0,
                        0.0, op0=mybir.AluOpType.mult,
                        op1=mybir.AluOpType.add)
                    state["tau32"] = tau32
                    state["cpos"] = cpos

                def s_cpos(lo, hi):
                    def f():
                        cpos, tau32 = state["cpos"], state["tau32"]
                        for gl in range(lo, hi):
                            nc.gpsimd.tensor_scalar(
                                cpos[:, gl, 1:], scg[:, gl, :],
                                tau32[:, gl, :], 1.0,
                                op0=mybir.AluOpType.is_ge,
                                op1=mybir.AluOpType.mult)
                    return f

                def s_scan():
                    cpos = state["cpos"]
                    P = ppool.tile([128, gb, Wc + 1], f16, name=f"P{G}",
                                  tag="P")
                    nc.vector.tensor_tensor_scan(
                        P.rearrange("p a b -> p (a b)"),
                        gate_cls[Wc][:, 0:gb, :].rearrange("p a b -> p (a b)"),
                        cpos.rearrange("p a b -> p (a b)"),
                        0.0, op0=mybir.AluOpType.mult,
                        op1=mybir.AluOpType.add)
                    state["P"] = P

                def s_u():
                    u = upool.tile([128, gb, Wc + 1], f16, name=f"u{G}",
                                   tag="u")
                    nc.vector.tensor_scalar(
                        u[:, :, :], state["cpos"][:, :, :], BIGC, 0.0,
                        op0=mybir.AluOpType.mult, op1=mybir.AluOpType.add)
                    state["u"] = u

                def s_idx():
                    idx = ixpool.tile([128, gb * Wc], i16, name=f"ix{G}",
                                    tag="idx")
                    nc.vector.tensor_tensor(
                        idx[:, :].rearrange("p (a b) -> p a b", a=gb),
                        state["u"][:, :, 1:], state["P"][:, :, 1:],
                        mybir.AluOpType.add)
                    state["idx"] = idx

                def s_scatter():
                    dst = dstp.tile([128, gb * BINS], i16, name=f"d{G}",
                                    tag="dst")
                    nc.gpsimd.local_scatter(
                        dst[:, :], iota_cls[Wc][:, 0:gb * Wc],
                        state["idx"][:, :],
                        channels=128, num_elems=gb * BINS, num_idxs=gb * Wc)
                    state["dst"] = dst

                def s_out():
                    dview = state["dst"][:, :].rearrange(
                        "p (a b) -> p a b", a=gb)
                    c0 = GRP_COL0[G]
                    nc.vector.tensor_tensor(
                        dview[:, :, 16:17],
                        dview[:, :, 16:17],
                        loc1_sb[:, c0:c0 + gb].unsqueeze(2),
                        mybir.AluOpType.min)
                    jb = b * TPB + min(i for _, i in tiles)
                    nc.sync.dma_start(out=out_v[:, jb:jb + gb, :],
                                      in_=dview[:, :, 1:17])

                s_tau()
                if gb == GB:
                    return [s_cpos(0, 7), s_cpos(7, gb), None, None, None,
                            None, None, None, s_u, s_scan, s_idx, s_scatter,
                            None, s_out]
                h = min(4, gb)
                steps = [s_cpos(0, h)]
                if gb > h:
                    steps.append(s_cpos(h, gb))
                steps += [None, s_u, s_scan, s_idx, s_scatter, s_out]
                return steps

            pending = []
            for G, (s0, gb) in enumerate(GRPS):
                tiles = TILES[s0:s0 + gb]
                Wc = GRP_WMAX[G]
                scg = scgp.tile([128, gb, Wc], f16, name=f"scg{G}", tag="scg")
                vgrp = vpool.tile([128, gb, SEL], f16, name=f"v{G}", tag="v")
                for gl, (bb, i) in enumerate(tiles):
                    W = 2 * i + 2
                    colbase = bb * T + i * 128
                    ps = sc_ps.tile([128, Wc], f32, name=f"ps{G}_{gl}",
                                    tag="scps")
                    nc.tensor.matmul(ps, lhsT=qT_sb[:, colbase:colbase + 128],
                                     rhs=M_sb[bb][:, 0:Wc],
                                     start=True, stop=False)
                    nc.tensor.matmul(ps[:, 2 * i + 1:2 * i + 2],
                                     lhsT=zap_sb, rhs=one_sb,
                                     start=False, stop=True)
                    nc.scalar.copy(out=scg[:, gl, :], in_=ps)
                    sc2 = small.tile([128, W], f16, name=f"sc2_{G}_{gl}",
                                     tag="sc2")
                    va = vgrp[:, gl, 0:8]
                    vb = vgrp[:, gl, 8:16]
                    nc.vector.max(out=va, in_=scg[:, gl, 0:W])
                    nc.vector.match_replace(out=sc2, in_to_replace=va,
                                            in_values=scg[:, gl, 0:W],
                                            imm_value=ZAPV)
                    nc.vector.max(out=vb, in_=sc2)
                    if pending:
                        s = pending.pop(0)
                        if s is not None:
                            s()
                for s in pending:       # drain any leftovers at group end
                    if s is not None:
                        s()
                if G == 1:
                    make_m(1)
                pending = extraction_steps(G, scg, vgrp)
            for s in pending:
                if s is not None:
                    s()

    nc.compile()
    return nc


def _shard_inputs(Q, K, Wq, Wk):
    early, loc1, pf, pr, zap, one, blkind = _static_tables()
    in_maps = []
    for h in range(H):
        qT = np.ascontiguousarray(
            Q[:, :, GROUPS * h, :].reshape(ROWS, D).T).astype(np.float16)
        kcs = {}
        for b in range(B):
            kb = K[b, :, h, :].reshape(64, 128, D).transpose(1, 0, 2)
            kcs[f"kc{b}"] = np.ascontiguousarray(
                kb.reshape(128, 64 * D)).astype(np.float16)
        G = (Wq[h].astype(np.float64)
             @ Wk[h].astype(np.float64).T / BS).astype(np.float32)
        bun16 = np.hstack([blkind, pf, pr]).astype(np.float16)
        zapone = np.hstack([zap, one]).astype(np.float16)
        buni = np.hstack([loc1, early]).astype(np.int16)
        in_maps.append({
            "qT": qT, **kcs,
            "gT": np.ascontiguousarray(G.T),
            "bun16": bun16, "zapone": zapone, "buni": buni,
        })
    return in_maps


def kernel(Q, K, Wq, Wk, logit_scale=None, block_size=64, selected_blocks=16,
           groups=4, **_unused):
    assert int(block_size) == BS and int(selected_blocks) == SEL
    assert int(groups) == GROUPS
    Q = np.asarray(Q, np.float32)
    K = np.asarray(K, np.float32)
    Wq = np.asarray(Wq, np.float32)
    Wk = np.asarray(Wk, np.float32)
    # exp(logit_scale) > 0 scales scores per-head only -> ranking unchanged.

    if "nc" not in _CACHE:
        _CACHE["nc"] = build_program()
    nc = _CACHE["nc"]

    in_maps = _shard_inputs(Q, K, Wq, Wk)
    res = run_bass_kernel_spmd(nc, in_maps, core_ids=list(range(H)))
    outs = [res.results[h]["out"] for h in range(H)]          # [ROWS, SEL] i16
    out = np.stack(outs, axis=1).reshape(B, T, H, SEL)
    return out.astype(np.int32)


if __name__ == "__main__":
    rng = np.random.default_rng(0)
    Q = rng.standard_normal((B, T, HQ, D)).astype(np.float32)
    K = rng.standard_normal((B, T, H, D)).astype(np.float32)
    Wq = (rng.standard_normal((H, D, DR)) * 0.02).astype(np.float32)
    Wk = (rng.standard_normal((H, D, DR)) * 0.02).astype(np.float32)
    out = kernel(Q=Q, K=K, Wq=Wq, Wk=Wk)
    print("kernel ran:", out.shape, out.dtype)


# revision 3
# speedup vs baseline: 1.0703x; 1.0039x over previous
"""Trainium2 Bass kernel v2 for nn_MiniDSARouter (topk block routing).

Shapes: B=2, T=8192, HQ=32, H=8, D=64, DR=16, block_size=64,
selected_blocks=16, groups=4, ADD_LOCAL=1. One KV head per core.

Semantics (same reduction as baseline, verified vs reference):
  out[b,t,h,:] = sorted_asc(top16_idx(scores[b,t,h,:]))
  with out[15] := min(out[15], t_blk-1)
where scores = q^T (Wq Wk^T/64) ksum^T with causal block mask, and
rows with t_blk <= 15 are a static function of t.

v2 pipeline per 128-row tile (vs baseline's 6 DVE passes + Act copy):
  PE:   scores = qT^T @ M in ONE fp16 matmul (M = G @ blocksum(K),
        G = Wq Wk^T/64 precomputed on host), plus fp16 "zap" matmul
        masking block 2i+1 for rows p<64.
  Act:  copy PSUM fp32 -> SBUF fp16 group buffer.
  DVE:  Max8 / MatchReplace / Max8 -> top-16 values, tau = v[15].
Then per group of 14 tiles (batched ops):
  cpos = (sc >= tau) via one TT is_ge (Pool), with a planted column per
  tile carrying 64*gl; one gated segmented scan (Pool) turns marks into
  per-column rank P + 64*gl; one scalar_tensor_tensor (DVE, 4x fp16)
  maps marked columns to unique bins P+64*gl and unmarked to negative;
  one batched gpsimd local_scatter writes column index j into bin
  rank-1 -- dst bins [64gl+1 .. 64gl+16] ARE the sorted top-16 indices.
  One TT min clamps slot 15 with t_blk-1. DMA out as int16.

All DRAM I/O is fp16/int16 (half the baseline's DMA traffic).
"""

import numpy as np

import concourse.bass as bass
import concourse.mybir as mybir
import concourse.tile as tile
from concourse import bacc
from concourse.bass_utils import run_bass_kernel_spmd

B, T, HQ, H, D, DR = 2, 8192, 32, 8, 64, 16
BS = 64
NB = T // BS               # 128 blocks per batch
SEL = 16
GROUPS = 4
ROWS = B * T               # 16384 rows per core
NTILES_SKIP = 8            # per-batch tiles 0..7 (t < 1024) are static
TPB = T // 128             # 64 row-tiles per batch
NGT = TPB - NTILES_SKIP    # 56 computed tiles per batch
NG = B * NGT               # 112 computed tiles per core
GB = 14                    # tiles per group
NGRP = NG // GB            # 8 groups
BINS = 64                  # scatter bins per tile
BIGC = 1024.0              # unmarked-to-negative shift
ZAPV = -60000.0

_CACHE = {}


def _tiles():
    """b0 ascending i, then b1 descending i (drain on cheap tiles)."""
    return ([(0, NTILES_SKIP + k) for k in range(NGT)]
            + [(1, TPB - 1 - k) for k in range(NGT)])


TILES = _tiles()
# groups: 7x14 tiles + 2x7 (short tail groups drain the pipeline fast)
GRPS = [(14 * k, 14) for k in range(7)] + [(98, 7), (105, 7)]
GRP_WMAX = [max(2 * i + 2 for _, i in TILES[s:s + n]) for s, n in GRPS]
CLASSES = sorted(set(GRP_WMAX))                      # [44, 72, 100, 128]


def _static_tables():
    # early rows: t_blk <= 15 -> sorted([0..15] + [t_blk, max(t_blk-1,0)])[:16]
    early = np.empty((128, NTILES_SKIP, SEL), np.int16)
    for t in range(NTILES_SKIP * 128):
        tb = t // BS
        s = sorted(list(range(16)) + [tb, max(tb - 1, 0)])
        early[t % 128, t // 128] = s[:SEL]
    early = early.reshape(128, NTILES_SKIP * SEL)

    # plants: per-tile scan seed = 64*outpos - BIGC, so the stt's
    # (cpos*BIGC + P') lands marked cols at bin 64*outpos + rank and
    # unmarked cols strictly negative. fwd for b0 groups, rev for b1.
    pf = np.tile((np.arange(GB) * BINS - BIGC).astype(np.float16), (128, 1))
    pr = pf[:, ::-1].copy()
    zap = np.zeros((1, 128), np.float16)
    zap[0, :64] = ZAPV
    one = np.ones((1, 1), np.float16)
    blkind = np.zeros((128, 2), np.float16)
    blkind[:64, 0] = 1.0
    blkind[64:, 1] = 1.0
    return early, pf, pr, zap, one, blkind


def build_program():
    f32 = mybir.dt.float32
    f16 = mybir.dt.float16
    i16 = mybir.dt.int16
    nc = bacc.Bacc("TRN2", target_bir_lowering=False, debug=False)

    qT_d = nc.dram_tensor("qT", [D, ROWS], f16, kind="ExternalInput")
    kc_d = [nc.dram_tensor(f"kc{b}", [128, 64 * D], f16, kind="ExternalInput")
            for b in range(B)]
    gT_d = nc.dram_tensor("gT", [D, D], f32, kind="ExternalInput")
    bun16_d = nc.dram_tensor("bun16", [128, 2 + 2 * GB], f16,
                             kind="ExternalInput")
    zapone_d = nc.dram_tensor("zapone", [1, 129], f16, kind="ExternalInput")
    buni_d = nc.dram_tensor("buni", [128, NTILES_SKIP * SEL], i16,
                            kind="ExternalInput")
    out_d = nc.dram_tensor("out", [ROWS, SEL], i16, kind="ExternalOutput")

    with tile.TileContext(nc) as tc:
        with (
            tc.tile_pool(name="singles", bufs=1) as singles,
            tc.tile_pool(name="qchunk", bufs=1) as qpool,
            tc.tile_pool(name="sc_ps", bufs=8, space="PSUM") as sc_ps,
            tc.tile_pool(name="scg", bufs=4) as scgp,
            tc.tile_pool(name="small", bufs=8) as small,
            tc.tile_pool(name="vpool", bufs=3) as vpool,
            tc.tile_pool(name="taupool", bufs=3) as taupool,
            tc.tile_pool(name="cpool", bufs=2) as cpool,
            tc.tile_pool(name="ppool", bufs=2) as ppool,
            tc.tile_pool(name="upool", bufs=2) as upool,
            tc.tile_pool(name="ixpool", bufs=2) as ixpool,
            tc.tile_pool(name="dstp", bufs=3) as dstp,
        ):
            # ---------------- static tables / params ----------------
            out_v = out_d.ap().rearrange("(j p) s -> p j s", p=128)
            warm = singles.tile([1, 2], f16)
            nc.vector.memset(warm, 0.0)
            warm2 = singles.tile([1, 2], f16)
            nc.scalar.copy(out=warm2, in_=warm)

            # K chunks + blocksum matmuls + M per batch
            kc_sb = [singles.tile([128, 64 * D], f16, name=f"kcs{b}")
                     for b in range(B)]
            M_sb = [singles.tile([D, NB], f16, name=f"Msb{b}")
                    for b in range(B)]
            qT_sb = singles.tile([D, ROWS], f16)
            KQ = 16 * D   # 16 chunks per DMA piece -> 4 pieces per batch

            def load_kc(b, pieces=1):
                n = (64 * D) // pieces
                for q in range(pieces):
                    nc.sync.dma_start(out=kc_sb[b][:, q * n:(q + 1) * n],
                                      in_=kc_d[b].ap()[:, q * n:(q + 1) * n])

            def make_m(b):
                kp = sc_ps.tile([D, NB], f32, name=f"kps{b}", tag="scps")
                for c in range(64):
                    nc.tensor.matmul(kp[:, 2 * c:2 * c + 2],
                                     lhsT=kc_sb[b][:, c * D:(c + 1) * D],
                                     rhs=blkind_sb,
                                     start=True, stop=True)
                ks = small.tile([D, NB], f32, name=f"ksum{b}", tag="ksum")
                nc.scalar.copy(out=ks, in_=kp)
                mp = sc_ps.tile([D, NB], f32, name=f"mps{b}", tag="scps")
                nc.tensor.matmul(mp, lhsT=gT_sb, rhs=ks, start=True, stop=True)
                nc.scalar.copy(out=M_sb[b], in_=mp)

            # critical-path DMA order: kc0, first q chunk, small bundles,
            # kc1, rest of q, index tables
            load_kc(0, pieces=2)
            bun16_sb = singles.tile([128, 2 + 2 * GB], f16)
            nc.sync.dma_start(out=bun16_sb, in_=bun16_d.ap())
            nc.sync.dma_start(out=qT_sb[:, 1024:3072],
                              in_=qT_d.ap()[:, 1024:3072])
            gT_sb = singles.tile([D, D], f32)
            nc.sync.dma_start(out=gT_sb, in_=gT_d.ap())
            zapone_sb = singles.tile([1, 129], f16)
            nc.sync.dma_start(out=zapone_sb, in_=zapone_d.ap())
            blkind_sb = bun16_sb[:, 0:2]
            pf_sb = bun16_sb[:, 2:2 + GB]
            pr_sb = bun16_sb[:, 2 + GB:2 + 2 * GB]
            zap_sb = zapone_sb[:, 0:128]
            one_sb = zapone_sb[:, 128:129]
            make_m(0)
            load_kc(1)
            qranges = [(3072, 5120), (5120, 8192), (12288, 16384),
                       (8192, 12288)]
            for lo, hi in qranges:
                nc.sync.dma_start(out=qT_sb[:, lo:hi],
                                  in_=qT_d.ap()[:, lo:hi])

            buni_sb = singles.tile([128, NTILES_SKIP * SEL], i16)
            nc.sync.dma_start(out=buni_sb, in_=buni_d.ap())
            early_v = buni_sb[:, :].rearrange(
                "p (a b) -> p a b", a=NTILES_SKIP)

            def emit_early_out():
                for b in range(B):
                    jb = b * TPB
                    nc.sync.dma_start(out=out_v[:, jb:jb + NTILES_SKIP, :],
                                      in_=early_v)

            # gate + iota masters per width class (run during the DMA wait)
            gate_cls = {}
            iota_cls = {}
            for Wc in CLASSES:
                gt = singles.tile([128, GB, Wc + 1], f16, name=f"gate{Wc}")
                nc.gpsimd.memset(gt.rearrange("p a b -> p (a b)"), 1.0)
                nc.gpsimd.memset(gt[:, :, 0:1], 0.0)
                gate_cls[Wc] = gt
                it = singles.tile([128, GB * Wc], i16, name=f"iota{Wc}")
                nc.gpsimd.iota(it[:, :], pattern=[[0, GB], [1, Wc]],
                               base=0, channel_multiplier=0)
                iota_cls[Wc] = it

            # ---------------- main loop (software-pipelined) ----------------
            GRP_COL0 = [sum(n for _, n in GRPS[:G]) for G in range(len(GRPS))]

            def extraction_steps(G, scg, vgrp):
                """Deferred per-group extraction, emitted during group G+1's
                tile loop so the in-order DVE/Pool queues never head-of-line
                block on cross-engine dependencies."""
                s0, gb = GRPS[G]
                tiles = TILES[s0:s0 + gb]
                b = tiles[0][0]
                rev = tiles[0][1] > tiles[-1][1]
                Wc = GRP_WMAX[G]
                # plant slice: fwd groups use pf[:gb]; rev use pr tail
                plant = (pr_sb[:, GB - gb:GB] if rev else pf_sb[:, 0:gb])
                state = {}

                def s_tau():
                    tau32 = taupool.tile([128, gb, 1], mybir.dt.float32,
                                      name=f"tau{G}", tag="tau32")
                    nc.vector.tensor_scalar(tau32, vgrp[:, :, 15:16], 1.0,
                                            0.0, op0=mybir.AluOpType.mult,
                                            op1=mybir.AluOpType.add)
                    cpos = cpool.tile([128, gb, Wc + 1], f16, name=f"cp{G}",
                                     tag="cpos")
                    nc.vector.tensor_scalar(
                        cpos[:, :, 0:1], plant.unsqueeze(2), 1.0,
                        0.0, op0=mybir.AluOpType.mult,
                        op1=mybir.AluOpType.add)
                    state["tau32"] = tau32
                    state["cpos"] = cpos

                def s_cpos(lo, hi, dve=False):
                    eng = nc.vector if dve else nc.gpsimd

                    def f():
                        cpos, tau32 = state["cpos"], state["tau32"]
                        for gl in range(lo, hi):
                            eng.tensor_scalar(
                                cpos[:, gl, 1:], scg[:, gl, :],
                                tau32[:, gl, :], 1.0,
                                op0=mybir.AluOpType.is_ge,
                                op1=mybir.AluOpType.mult)
                    return f

                def s_scan():
                    cpos = state["cpos"]
                    P = ppool.tile([128, gb, Wc + 1], f16, name=f"P{G}",
                                  tag="P")
                    nc.vector.tensor_tensor_scan(
                        P.rearrange("p a b -> p (a b)"),
                        gate_cls[Wc][:, 0:gb, :].rearrange("p a b -> p (a b)"),
                        cpos.rearrange("p a b -> p (a b)"),
                        0.0, op0=mybir.AluOpType.mult,
                        op1=mybir.AluOpType.add)
                    state["P"] = P

                def s_u():
                    u = upool.tile([128, gb, Wc + 1], f16, name=f"u{G}",
                                   tag="u")
                    nc.vector.tensor_scalar(
                        u[:, :, :], state["cpos"][:, :, :], BIGC, 0.0,
                        op0=mybir.AluOpType.mult, op1=mybir.AluOpType.add)
                    state["u"] = u

                def s_idx():
                    idx = ixpool.tile([128, gb * Wc], i16, name=f"ix{G}",
                                    tag="idx")
                    nc.vector.tensor_tensor(
                        idx[:, :].rearrange("p (a b) -> p a b", a=gb),
                        state["u"][:, :, 1:], state["P"][:, :, 1:],
                        mybir.AluOpType.add)
                    state["idx"] = idx

                def s_scatter():
                    dst = dstp.tile([128, gb * BINS], i16, name=f"d{G}",
                                    tag="dst")
                    nc.gpsimd.local_scatter(
                        dst[:, :], iota_cls[Wc][:, 0:gb * Wc],
                        state["idx"][:, :],
                        channels=128, num_elems=gb * BINS, num_idxs=gb * Wc)
                    state["dst"] = dst

                def s_out():
                    dview = state["dst"][:, :].rearrange(
                        "p (a b) -> p a b", a=gb)
                    jb = b * TPB + min(i for _, i in tiles)
                    nc.sync.dma_start(out=out_v[:, jb:jb + gb, :],
                                      in_=dview[:, :, 1:17])

                s_tau()
                if gb == GB:
                    return [s_cpos(0, 7), s_cpos(7, gb, dve=rev), None, None,
                            None, None, None, None, s_u, s_scan, s_idx,
                            s_scatter, s_out]
                h = min(4, gb)
                steps = [s_cpos(0, h)]
                if gb > h:
                    steps.append(s_cpos(h, gb, dve=rev))
                steps += [None, s_u, s_scan, s_idx, s_scatter, s_out]
                return steps

            pending = []
            for G, (s0, gb) in enumerate(GRPS):
                tiles = TILES[s0:s0 + gb]
                Wc = GRP_WMAX[G]
                scg = scgp.tile([128, gb, Wc], f16, name=f"scg{G}", tag="scg")
                vgrp = vpool.tile([128, gb, SEL], f16, name=f"v{G}", tag="v")
                for gl, (bb, i) in enumerate(tiles):
                    W = 2 * i + 2
                    colbase = bb * T + i * 128
                    ps = sc_ps.tile([128, Wc], f32, name=f"ps{G}_{gl}",
                                    tag="scps")
                    nc.tensor.matmul(ps, lhsT=qT_sb[:, colbase:colbase + 128],
                                     rhs=M_sb[bb][:, 0:Wc],
                                     start=True, stop=False)
                    nc.tensor.matmul(ps[:, 2 * i + 1:2 * i + 2],
                                     lhsT=zap_sb, rhs=one_sb,
                                     start=False, stop=True)
                    nc.scalar.copy(out=scg[:, gl, :], in_=ps)
                    sc2 = small.tile([128, W], f16, name=f"sc2_{G}_{gl}",
                                     tag="sc2")
                    va = vgrp[:, gl, 0:8]
                    vb = vgrp[:, gl, 8:16]
                    nc.vector.max(out=va, in_=scg[:, gl, 0:W])
                    nc.vector.match_replace(out=sc2, in_to_replace=va,
                                            in_values=scg[:, gl, 0:W],
                                            imm_value=ZAPV)
                    nc.vector.max(out=vb, in_=sc2)
                    if pending:
                        s = pending.pop(0)
                        if s is not None:
                            s()
                for s in pending:       # drain any leftovers at group end
                    if s is not None:
                        s()
                if G == 1:
                    make_m(1)
                    emit_early_out()
                pending = extraction_steps(G, scg, vgrp)
            for s in pending:
                if s is not None:
                    s()

    nc.compile()
    return nc


def _shard_inputs(Q, K, Wq, Wk):
    early, pf, pr, zap, one, blkind = _static_tables()
    in_maps = []
    for h in range(H):
        qT = np.ascontiguousarray(
            Q[:, :, GROUPS * h, :].reshape(ROWS, D).T).astype(np.float16)
        kcs = {}
        for b in range(B):
            kb = K[b, :, h, :].reshape(64, 128, D).transpose(1, 0, 2)
            kcs[f"kc{b}"] = np.ascontiguousarray(
                kb.reshape(128, 64 * D)).astype(np.float16)
        G = (Wq[h].astype(np.float64)
             @ Wk[h].astype(np.float64).T / BS).astype(np.float32)
        bun16 = np.hstack([blkind, pf, pr]).astype(np.float16)
        zapone = np.hstack([zap, one]).astype(np.float16)
        buni = early.astype(np.int16)
        in_maps.append({
            "qT": qT, **kcs,
            "gT": np.ascontiguousarray(G.T),
            "bun16": bun16, "zapone": zapone, "buni": buni,
        })
    return in_maps


def kernel(Q, K, Wq, Wk, logit_scale=None, block_size=64, selected_blocks=16,
           groups=4, **_unused):
    assert int(block_size) == BS and int(selected_blocks) == SEL
    assert int(groups) == GROUPS
    Q = np.asarray(Q, np.float32)
    K = np.asarray(K, np.float32)
    Wq = np.asarray(Wq, np.float32)
    Wk = np.asarray(Wk, np.float32)
    # exp(logit_scale) > 0 scales scores per-head only -> ranking unchanged.

    if "nc" not in _CACHE:
        _CACHE["nc"] = build_program()
    nc = _CACHE["nc"]

    in_maps = _shard_inputs(Q, K, Wq, Wk)
    res = run_bass_kernel_spmd(nc, in_maps, core_ids=list(range(H)))
    outs = [res.results[h]["out"] for h in range(H)]          # [ROWS, SEL] i16
    out = np.stack(outs, axis=1).reshape(B, T, H, SEL).astype(np.int32)
    # union-with-locals clamp: out[..., 15] = min(out[..., 15], t_blk - 1)
    # (early rows t < 1024 come from the static table and are left as-is)
    tbm1 = np.maximum(np.arange(T) // BS - 1, 0).astype(np.int32)
    out[:, 1024:, :, 15] = np.minimum(out[:, 1024:, :, 15],
                                      tbm1[1024:, None])
    return out


if __name__ == "__main__":
    rng = np.random.default_rng(0)
    Q = rng.standard_normal((B, T, HQ, D)).astype(np.float32)
    K = rng.standard_normal((B, T, H, D)).astype(np.float32)
    Wq = (rng.standard_normal((H, D, DR)) * 0.02).astype(np.float32)
    Wk = (rng.standard_normal((H, D, DR)) * 0.02).astype(np.float32)
    out = kernel(Q=Q, K=K, Wq=Wq, Wk=Wk)
    print("kernel ran:", out.shape, out.dtype)


# revision 4
# speedup vs baseline: 1.0791x; 1.0082x over previous
"""Trainium2 Bass kernel v2 for nn_MiniDSARouter (topk block routing).

Shapes: B=2, T=8192, HQ=32, H=8, D=64, DR=16, block_size=64,
selected_blocks=16, groups=4, ADD_LOCAL=1. One KV head per core.

Semantics (same reduction as baseline, verified vs reference):
  out[b,t,h,:] = sorted_asc(top16_idx(scores[b,t,h,:]))
  with out[15] := min(out[15], t_blk-1)
where scores = q^T (Wq Wk^T/64) ksum^T with causal block mask, and
rows with t_blk <= 15 are a static function of t.

v2 pipeline per 128-row tile (vs baseline's 6 DVE passes + Act copy):
  PE:   scores = qT^T @ M in ONE fp16 matmul (M = G @ blocksum(K),
        G = Wq Wk^T/64 precomputed on host), plus fp16 "zap" matmul
        masking block 2i+1 for rows p<64.
  Act:  copy PSUM fp32 -> SBUF fp16 group buffer.
  DVE:  Max8 / MatchReplace / Max8 -> top-16 values, tau = v[15].
Then per group of 14 tiles (batched ops):
  cpos = (sc >= tau) via one TT is_ge (Pool), with a planted column per
  tile carrying 64*gl; one gated segmented scan (Pool) turns marks into
  per-column rank P + 64*gl; one scalar_tensor_tensor (DVE, 4x fp16)
  maps marked columns to unique bins P+64*gl and unmarked to negative;
  one batched gpsimd local_scatter writes column index j into bin
  rank-1 -- dst bins [64gl+1 .. 64gl+16] ARE the sorted top-16 indices.
  One TT min clamps slot 15 with t_blk-1. DMA out as int16.

All DRAM I/O is fp16/int16 (half the baseline's DMA traffic).
"""

import numpy as np

import concourse.bass as bass
import concourse.mybir as mybir
import concourse.tile as tile
from concourse import bacc
from concourse.bass_utils import run_bass_kernel_spmd

B, T, HQ, H, D, DR = 2, 8192, 32, 8, 64, 16
BS = 64
NB = T // BS               # 128 blocks per batch
SEL = 16
GROUPS = 4
ROWS = B * T               # 16384 rows per core
NTILES_SKIP = 8            # per-batch tiles 0..7 (t < 1024) are static
TPB = T // 128             # 64 row-tiles per batch
NGT = TPB - NTILES_SKIP    # 56 computed tiles per batch
NG = B * NGT               # 112 computed tiles per core
GB = 14                    # tiles per group
NGRP = NG // GB            # 8 groups
BINS = 64                  # scatter bins per tile
BIGC = 1024.0              # unmarked-to-negative shift
ZAPV = -60000.0

_CACHE = {}


def _tiles():
    """b0 ascending i, then b1 descending i (drain on cheap tiles)."""
    return ([(0, NTILES_SKIP + k) for k in range(NGT)]
            + [(1, TPB - 1 - k) for k in range(NGT)])


TILES = _tiles()
# groups: 7x14 tiles + 2x7 (short tail groups drain the pipeline fast)
GRPS = [(14 * k, 14) for k in range(7)] + [(98, 7), (105, 7)]
GRP_WMAX = [max(2 * i + 2 for _, i in TILES[s:s + n]) for s, n in GRPS]
CLASSES = sorted(set(GRP_WMAX))                      # [44, 72, 100, 128]


def _static_tables():
    # early rows: t_blk <= 15 -> sorted([0..15] + [t_blk, max(t_blk-1,0)])[:16]
    early = np.empty((128, NTILES_SKIP, SEL), np.int16)
    for t in range(NTILES_SKIP * 128):
        tb = t // BS
        s = sorted(list(range(16)) + [tb, max(tb - 1, 0)])
        early[t % 128, t // 128] = s[:SEL]
    early = early.reshape(128, NTILES_SKIP * SEL)

    # plants: per-tile scan seed = 64*outpos - BIGC, so the stt's
    # (cpos*BIGC + P') lands marked cols at bin 64*outpos + rank and
    # unmarked cols strictly negative. fwd for b0 groups, rev for b1.
    pf = np.tile((np.arange(GB) * BINS - BIGC).astype(np.float16), (128, 1))
    pr = pf[:, ::-1].copy()
    zap = np.zeros((1, 128), np.float16)
    zap[0, :64] = ZAPV
    one = np.ones((1, 1), np.float16)
    blkind = np.zeros((128, 2), np.float16)
    blkind[:64, 0] = 1.0
    blkind[64:, 1] = 1.0
    return early, pf, pr, zap, one, blkind


def build_program():
    f32 = mybir.dt.float32
    f16 = mybir.dt.float16
    i16 = mybir.dt.int16
    nc = bacc.Bacc("TRN2", target_bir_lowering=False, debug=False)

    qT_d = nc.dram_tensor("qT", [D, ROWS], f16, kind="ExternalInput")
    kc_d = [nc.dram_tensor(f"kc{b}", [128, 64 * D], f16, kind="ExternalInput")
            for b in range(B)]
    gT_d = nc.dram_tensor("gT", [D, D], f32, kind="ExternalInput")
    bun16_d = nc.dram_tensor("bun16", [128, 2 + 2 * GB], f16,
                             kind="ExternalInput")
    zapone_d = nc.dram_tensor("zapone", [1, 129], f16, kind="ExternalInput")
    buni_d = nc.dram_tensor("buni", [128, NTILES_SKIP * SEL], i16,
                            kind="ExternalInput")
    out_d = nc.dram_tensor("out", [ROWS, SEL], i16, kind="ExternalOutput")

    with tile.TileContext(nc) as tc:
        with (
            tc.tile_pool(name="singles", bufs=1) as singles,
            tc.tile_pool(name="qchunk", bufs=1) as qpool,
            tc.tile_pool(name="sc_ps", bufs=8, space="PSUM") as sc_ps,
            tc.tile_pool(name="scg", bufs=4) as scgp,
            tc.tile_pool(name="small", bufs=8) as small,
            tc.tile_pool(name="vpool", bufs=3) as vpool,
            tc.tile_pool(name="taupool", bufs=3) as taupool,
            tc.tile_pool(name="cpool", bufs=2) as cpool,
            tc.tile_pool(name="ppool", bufs=2) as ppool,
            tc.tile_pool(name="upool", bufs=2) as upool,
            tc.tile_pool(name="ixpool", bufs=2) as ixpool,
            tc.tile_pool(name="dstp", bufs=3) as dstp,
        ):
            # ---------------- static tables / params ----------------
            out_v = out_d.ap().rearrange("(j p) s -> p j s", p=128)
            warm = singles.tile([1, 2], f16)
            nc.vector.memset(warm, 0.0)
            warm2 = singles.tile([1, 2], f16)
            nc.scalar.copy(out=warm2, in_=warm)

            # K chunks + blocksum matmuls + M per batch
            kc_sb = [singles.tile([128, 64 * D], f16, name=f"kcs{b}")
                     for b in range(B)]
            M_sb = [singles.tile([D, NB], f16, name=f"Msb{b}")
                    for b in range(B)]
            qT_sb = singles.tile([D, ROWS], f16)
            KQ = 16 * D   # 16 chunks per DMA piece -> 4 pieces per batch

            def load_kc(b, pieces=1):
                n = (64 * D) // pieces
                for q in range(pieces):
                    nc.sync.dma_start(out=kc_sb[b][:, q * n:(q + 1) * n],
                                      in_=kc_d[b].ap()[:, q * n:(q + 1) * n])

            def make_m(b):
                kp = sc_ps.tile([D, NB], f32, name=f"kps{b}", tag="scps")
                for c in range(64):
                    nc.tensor.matmul(kp[:, 2 * c:2 * c + 2],
                                     lhsT=kc_sb[b][:, c * D:(c + 1) * D],
                                     rhs=blkind_sb,
                                     start=True, stop=True)
                ks = small.tile([D, NB], f32, name=f"ksum{b}", tag="ksum")
                nc.scalar.copy(out=ks, in_=kp)
                mp = sc_ps.tile([D, NB], f32, name=f"mps{b}", tag="scps")
                nc.tensor.matmul(mp, lhsT=gT_sb, rhs=ks, start=True, stop=True)
                nc.scalar.copy(out=M_sb[b], in_=mp)

            # critical-path DMA order: kc0, first q chunk, small bundles,
            # kc1, rest of q, index tables
            load_kc(0, pieces=2)
            bun16_sb = singles.tile([128, 2 + 2 * GB], f16)
            nc.sync.dma_start(out=bun16_sb, in_=bun16_d.ap())
            nc.sync.dma_start(out=qT_sb[:, 1024:3072],
                              in_=qT_d.ap()[:, 1024:3072])
            gT_sb = singles.tile([D, D], f32)
            nc.sync.dma_start(out=gT_sb, in_=gT_d.ap())
            zapone_sb = singles.tile([1, 129], f16)
            nc.sync.dma_start(out=zapone_sb, in_=zapone_d.ap())
            blkind_sb = bun16_sb[:, 0:2]
            pf_sb = bun16_sb[:, 2:2 + GB]
            pr_sb = bun16_sb[:, 2 + GB:2 + 2 * GB]
            zap_sb = zapone_sb[:, 0:128]
            one_sb = zapone_sb[:, 128:129]
            make_m(0)
            load_kc(1)
            qranges = [(3072, 5120), (5120, 8192), (12288, 16384),
                       (8192, 12288)]
            for lo, hi in qranges:
                nc.sync.dma_start(out=qT_sb[:, lo:hi],
                                  in_=qT_d.ap()[:, lo:hi])

            buni_sb = singles.tile([128, NTILES_SKIP * SEL], i16)
            nc.sync.dma_start(out=buni_sb, in_=buni_d.ap())
            early_v = buni_sb[:, :].rearrange(
                "p (a b) -> p a b", a=NTILES_SKIP)

            def emit_early_out():
                for b in range(B):
                    jb = b * TPB
                    nc.sync.dma_start(out=out_v[:, jb:jb + NTILES_SKIP, :],
                                      in_=early_v)

            # gate + iota masters per width class (run during the DMA wait)
            gate_cls = {}
            iota_cls = {}
            for Wc in CLASSES:
                gt = singles.tile([128, GB, Wc + 1], f16, name=f"gate{Wc}")
                nc.gpsimd.memset(gt.rearrange("p a b -> p (a b)"), 1.0)
                nc.gpsimd.memset(gt[:, :, 0:1], 0.0)
                gate_cls[Wc] = gt
                it = singles.tile([128, GB * Wc], i16, name=f"iota{Wc}")
                nc.gpsimd.iota(it[:, :], pattern=[[0, GB], [1, Wc]],
                               base=0, channel_multiplier=0)
                iota_cls[Wc] = it

            # ---------------- main loop (software-pipelined) ----------------
            GRP_COL0 = [sum(n for _, n in GRPS[:G]) for G in range(len(GRPS))]

            def extraction_steps(G, scg, vgrp):
                """Deferred per-group extraction, emitted during group G+1's
                tile loop so the in-order DVE/Pool queues never head-of-line
                block on cross-engine dependencies."""
                s0, gb = GRPS[G]
                tiles = TILES[s0:s0 + gb]
                b = tiles[0][0]
                rev = tiles[0][1] > tiles[-1][1]
                Wc = GRP_WMAX[G]
                # plant slice: fwd groups use pf[:gb]; rev use pr tail
                plant = (pr_sb[:, GB - gb:GB] if rev else pf_sb[:, 0:gb])
                state = {}

                def s_tau():
                    tau32 = taupool.tile([128, gb, 1], mybir.dt.float32,
                                      name=f"tau{G}", tag="tau32")
                    nc.vector.tensor_scalar(tau32, vgrp[:, :, 15:16], 1.0,
                                            0.0, op0=mybir.AluOpType.mult,
                                            op1=mybir.AluOpType.add)
                    cpos = cpool.tile([128, gb, Wc + 1], f16, name=f"cp{G}",
                                     tag="cpos")
                    nc.vector.tensor_scalar(
                        cpos[:, :, 0:1], plant.unsqueeze(2), 1.0,
                        0.0, op0=mybir.AluOpType.mult,
                        op1=mybir.AluOpType.add)
                    state["tau32"] = tau32
                    state["cpos"] = cpos

                def s_cpos(lo, hi, dve=False):
                    eng = nc.vector if dve else nc.gpsimd

                    def f():
                        cpos, tau32 = state["cpos"], state["tau32"]
                        for gl in range(lo, hi):
                            eng.tensor_scalar(
                                cpos[:, gl, 1:], scg[:, gl, :],
                                tau32[:, gl, :], 1.0,
                                op0=mybir.AluOpType.is_ge,
                                op1=mybir.AluOpType.mult)
                    return f

                def s_scan():
                    cpos = state["cpos"]
                    P = ppool.tile([128, gb, Wc + 1], f16, name=f"P{G}",
                                  tag="P")
                    nc.vector.tensor_tensor_scan(
                        P.rearrange("p a b -> p (a b)"),
                        gate_cls[Wc][:, 0:gb, :].rearrange("p a b -> p (a b)"),
                        cpos.rearrange("p a b -> p (a b)"),
                        0.0, op0=mybir.AluOpType.mult,
                        op1=mybir.AluOpType.add)
                    state["P"] = P

                def s_u():
                    u = upool.tile([128, gb, Wc + 1], f16, name=f"u{G}",
                                   tag="u")
                    nc.vector.tensor_scalar(
                        u[:, :, :], state["cpos"][:, :, :], BIGC, 0.0,
                        op0=mybir.AluOpType.mult, op1=mybir.AluOpType.add)
                    state["u"] = u

                def s_idx():
                    idx = ixpool.tile([128, gb * Wc], i16, name=f"ix{G}",
                                    tag="idx")
                    nc.vector.tensor_tensor(
                        idx[:, :].rearrange("p (a b) -> p a b", a=gb),
                        state["u"][:, :, 1:], state["P"][:, :, 1:],
                        mybir.AluOpType.add)
                    state["idx"] = idx

                def s_scatter():
                    dst = dstp.tile([128, gb * BINS], i16, name=f"d{G}",
                                    tag="dst")
                    nc.gpsimd.local_scatter(
                        dst[:, :], iota_cls[Wc][:, 0:gb * Wc],
                        state["idx"][:, :],
                        channels=128, num_elems=gb * BINS, num_idxs=gb * Wc)
                    state["dst"] = dst

                def s_out():
                    dview = state["dst"][:, :].rearrange(
                        "p (a b) -> p a b", a=gb)
                    jb = b * TPB + min(i for _, i in tiles)
                    nc.sync.dma_start(out=out_v[:, jb:jb + gb, :],
                                      in_=dview[:, :, 1:17])

                if G == len(GRPS) - 1:
                    state["cpos"] = lcpos
                    return [s_u, s_scan, s_idx, s_scatter, s_out]
                s_tau()
                if gb == GB:
                    return [s_cpos(0, 7), s_cpos(7, gb, dve=rev), None, None,
                            None, None, None, None, s_u, s_scan, s_idx,
                            s_scatter, s_out]
                h = min(4, gb)
                steps = [s_cpos(0, h)]
                if gb > h:
                    steps.append(s_cpos(h, gb, dve=rev))
                steps += [None, s_u, s_scan, s_idx, s_scatter, s_out]
                return steps

            pending = []
            for G, (s0, gb) in enumerate(GRPS):
                tiles = TILES[s0:s0 + gb]
                Wc = GRP_WMAX[G]
                scg = scgp.tile([128, gb, Wc], f16, name=f"scg{G}", tag="scg")
                vgrp = vpool.tile([128, gb, SEL], f16, name=f"v{G}", tag="v")
                lastg = (G == len(GRPS) - 1)
                if lastg:
                    lcpos = cpool.tile([128, gb, Wc + 1], f16,
                                       name=f"lcp{G}", tag="cpos")
                    nc.vector.tensor_scalar(
                        lcpos[:, :, 0:1], pr_sb[:, GB - gb:GB].unsqueeze(2),
                        1.0, 0.0, op0=mybir.AluOpType.mult,
                        op1=mybir.AluOpType.add)
                for gl, (bb, i) in enumerate(tiles):
                    W = 2 * i + 2
                    colbase = bb * T + i * 128
                    ps = sc_ps.tile([128, Wc], f32, name=f"ps{G}_{gl}",
                                    tag="scps")
                    nc.tensor.matmul(ps, lhsT=qT_sb[:, colbase:colbase + 128],
                                     rhs=M_sb[bb][:, 0:Wc],
                                     start=True, stop=False)
                    nc.tensor.matmul(ps[:, 2 * i + 1:2 * i + 2],
                                     lhsT=zap_sb, rhs=one_sb,
                                     start=False, stop=True)
                    nc.scalar.copy(out=scg[:, gl, :], in_=ps)
                    sc2 = small.tile([128, W], f16, name=f"sc2_{G}_{gl}",
                                     tag="sc2")
                    va = vgrp[:, gl, 0:8]
                    vb = vgrp[:, gl, 8:16]
                    nc.vector.max(out=va, in_=scg[:, gl, 0:W])
                    nc.vector.match_replace(out=sc2, in_to_replace=va,
                                            in_values=scg[:, gl, 0:W],
                                            imm_value=ZAPV)
                    nc.vector.max(out=vb, in_=sc2)
                    if lastg:
                        ltau = taupool.tile([128, 1], mybir.dt.float32,
                                            name=f"lt{G}_{gl}", tag="ltau")
                        nc.vector.tensor_scalar(
                            ltau, vgrp[:, gl, 15:16], 1.0, 0.0,
                            op0=mybir.AluOpType.mult,
                            op1=mybir.AluOpType.add)
                        nc.vector.tensor_scalar(
                            lcpos[:, gl, 1:], scg[:, gl, :], ltau, 1.0,
                            op0=mybir.AluOpType.is_ge,
                            op1=mybir.AluOpType.mult)
                    if pending:
                        s = pending.pop(0)
                        if s is not None:
                            s()
                for s in pending:       # drain any leftovers at group end
                    if s is not None:
                        s()
                if G == 1:
                    make_m(1)
                    emit_early_out()
                pending = extraction_steps(G, scg, vgrp)
            for s in pending:
                if s is not None:
                    s()

    nc.compile()
    return nc


def _shard_inputs(Q, K, Wq, Wk):
    early, pf, pr, zap, one, blkind = _static_tables()
    in_maps = []
    for h in range(H):
        qT = np.ascontiguousarray(
            Q[:, :, GROUPS * h, :].reshape(ROWS, D).T).astype(np.float16)
        kcs = {}
        for b in range(B):
            kb = K[b, :, h, :].reshape(64, 128, D).transpose(1, 0, 2)
            kcs[f"kc{b}"] = np.ascontiguousarray(
                kb.reshape(128, 64 * D)).astype(np.float16)
        G = (Wq[h].astype(np.float64)
             @ Wk[h].astype(np.float64).T / BS).astype(np.float32)
        bun16 = np.hstack([blkind, pf, pr]).astype(np.float16)
        zapone = np.hstack([zap, one]).astype(np.float16)
        buni = early.astype(np.int16)
        in_maps.append({
            "qT": qT, **kcs,
            "gT": np.ascontiguousarray(G.T),
            "bun16": bun16, "zapone": zapone, "buni": buni,
        })
    return in_maps


def kernel(Q, K, Wq, Wk, logit_scale=None, block_size=64, selected_blocks=16,
           groups=4, **_unused):
    assert int(block_size) == BS and int(selected_blocks) == SEL
    assert int(groups) == GROUPS
    Q = np.asarray(Q, np.float32)
    K = np.asarray(K, np.float32)
    Wq = np.asarray(Wq, np.float32)
    Wk = np.asarray(Wk, np.float32)
    # exp(logit_scale) > 0 scales scores per-head only -> ranking unchanged.

    if "nc" not in _CACHE:
        _CACHE["nc"] = build_program()
    nc = _CACHE["nc"]

    in_maps = _shard_inputs(Q, K, Wq, Wk)
    res = run_bass_kernel_spmd(nc, in_maps, core_ids=list(range(H)))
    outs = [res.results[h]["out"] for h in range(H)]          # [ROWS, SEL] i16
    out = np.stack(outs, axis=1).reshape(B, T, H, SEL).astype(np.int32)
    # union-with-locals clamp: out[..., 15] = min(out[..., 15], t_blk - 1)
    # (early rows t < 1024 come from the static table and are left as-is)
    tbm1 = np.maximum(np.arange(T) // BS - 1, 0).astype(np.int32)
    out[:, 1024:, :, 15] = np.minimum(out[:, 1024:, :, 15],
                                      tbm1[1024:, None])
    return out


if __name__ == "__main__":
    rng = np.random.default_rng(0)
    Q = rng.standard_normal((B, T, HQ, D)).astype(np.float32)
    K = rng.standard_normal((B, T, H, D)).astype(np.float32)
    Wq = (rng.standard_normal((H, D, DR)) * 0.02).astype(np.float32)
    Wk = (rng.standard_normal((H, D, DR)) * 0.02).astype(np.float32)
    out = kernel(Q=Q, K=K, Wq=Wq, Wk=Wk)
    print("kernel ran:", out.shape, out.dtype)


# revision 5
# speedup vs baseline: 1.0848x; 1.0052x over previous
"""Trainium2 Bass kernel v2 for nn_MiniDSARouter (topk block routing).

Shapes: B=2, T=8192, HQ=32, H=8, D=64, DR=16, block_size=64,
selected_blocks=16, groups=4, ADD_LOCAL=1. One KV head per core.

Semantics (same reduction as baseline, verified vs reference):
  out[b,t,h,:] = sorted_asc(top16_idx(scores[b,t,h,:]))
  with out[15] := min(out[15], t_blk-1)
where scores = q^T (Wq Wk^T/64) ksum^T with causal block mask, and
rows with t_blk <= 15 are a static function of t.

v2 pipeline per 128-row tile (vs baseline's 6 DVE passes + Act copy):
  PE:   scores = qT^T @ M in ONE fp16 matmul (M = G @ blocksum(K),
        G = Wq Wk^T/64 precomputed on host), plus fp16 "zap" matmul
        masking block 2i+1 for rows p<64.
  Act:  copy PSUM fp32 -> SBUF fp16 group buffer.
  DVE:  Max8 / MatchReplace / Max8 -> top-16 values, tau = v[15].
Then per group of 14 tiles (batched ops):
  cpos = (sc >= tau) via one TT is_ge (Pool), with a planted column per
  tile carrying 64*gl; one gated segmented scan (Pool) turns marks into
  per-column rank P + 64*gl; one scalar_tensor_tensor (DVE, 4x fp16)
  maps marked columns to unique bins P+64*gl and unmarked to negative;
  one batched gpsimd local_scatter writes column index j into bin
  rank-1 -- dst bins [64gl+1 .. 64gl+16] ARE the sorted top-16 indices.
  One TT min clamps slot 15 with t_blk-1. DMA out as int16.

All DRAM I/O is fp16/int16 (half the baseline's DMA traffic).
"""

import numpy as np

import concourse.bass as bass
import concourse.mybir as mybir
import concourse.tile as tile
from concourse import bacc
from concourse.bass_utils import run_bass_kernel_spmd

B, T, HQ, H, D, DR = 2, 8192, 32, 8, 64, 16
BS = 64
NB = T // BS               # 128 blocks per batch
SEL = 16
GROUPS = 4
ROWS = B * T               # 16384 rows per core
NTILES_SKIP = 8            # per-batch tiles 0..7 (t < 1024) are static
TPB = T // 128             # 64 row-tiles per batch
NGT = TPB - NTILES_SKIP    # 56 computed tiles per batch
NG = B * NGT               # 112 computed tiles per core
GB = 14                    # tiles per group
NGRP = NG // GB            # 8 groups
BINS = 64                  # scatter bins per tile
BIGC = 1024.0              # unmarked-to-negative shift
ZAPV = -60000.0

_CACHE = {}


def _tiles():
    """b0 ascending i, then b1 descending i (drain on cheap tiles)."""
    return ([(0, NTILES_SKIP + k) for k in range(NGT)]
            + [(1, TPB - 1 - k) for k in range(NGT)])


TILES = _tiles()
# groups: 7x14 tiles + 2x7 (short tail groups drain the pipeline fast)
GRPS = [(14 * k, 14) for k in range(7)] + [(98, 7), (105, 7)]
GRP_WMAX = [max(2 * i + 2 for _, i in TILES[s:s + n]) for s, n in GRPS]
CLASSES = sorted(set(GRP_WMAX))                      # [44, 72, 100, 128]


def _static_tables():
    # early rows: t_blk <= 15 -> sorted([0..15] + [t_blk, max(t_blk-1,0)])[:16]
    early = np.empty((128, NTILES_SKIP, SEL), np.int16)
    for t in range(NTILES_SKIP * 128):
        tb = t // BS
        s = sorted(list(range(16)) + [tb, max(tb - 1, 0)])
        early[t % 128, t // 128] = s[:SEL]
    early = early.reshape(128, NTILES_SKIP * SEL)

    # plants: per-tile scan seed = 64*outpos - BIGC, so the stt's
    # (cpos*BIGC + P') lands marked cols at bin 64*outpos + rank and
    # unmarked cols strictly negative. fwd for b0 groups, rev for b1.
    pf = np.tile((np.arange(GB) * BINS - BIGC).astype(np.float16), (128, 1))
    pr = pf[:, ::-1].copy()
    zap = np.zeros((1, 128), np.float16)
    zap[0, :64] = ZAPV
    one = np.ones((1, 1), np.float16)
    blkind = np.zeros((128, 2), np.float16)
    blkind[:64, 0] = 1.0
    blkind[64:, 1] = 1.0
    return early, pf, pr, zap, one, blkind


def build_program():
    f32 = mybir.dt.float32
    f16 = mybir.dt.float16
    i16 = mybir.dt.int16
    nc = bacc.Bacc("TRN2", target_bir_lowering=False, debug=False)

    qT_d = nc.dram_tensor("qT", [D, ROWS], f16, kind="ExternalInput")
    kc_d = [nc.dram_tensor(f"kc{b}", [128, 64 * D], f16, kind="ExternalInput")
            for b in range(B)]
    gT_d = nc.dram_tensor("gT", [D, D], f32, kind="ExternalInput")
    bun16_d = nc.dram_tensor("bun16", [128, 2 + 2 * GB], f16,
                             kind="ExternalInput")
    zapone_d = nc.dram_tensor("zapone", [1, 129], f16, kind="ExternalInput")
    buni_d = nc.dram_tensor("buni", [128, NTILES_SKIP * SEL], i16,
                            kind="ExternalInput")
    out_d = nc.dram_tensor("out", [ROWS, SEL], i16, kind="ExternalOutput")

    with tile.TileContext(nc) as tc:
        with (
            tc.tile_pool(name="singles", bufs=1) as singles,
            tc.tile_pool(name="qchunk", bufs=1) as qpool,
            tc.tile_pool(name="sc_ps", bufs=8, space="PSUM") as sc_ps,
            tc.tile_pool(name="scg", bufs=4) as scgp,
            tc.tile_pool(name="small", bufs=8) as small,
            tc.tile_pool(name="vpool", bufs=3) as vpool,
            tc.tile_pool(name="taupool", bufs=3) as taupool,
            tc.tile_pool(name="cpool", bufs=2) as cpool,
            tc.tile_pool(name="ppool", bufs=2) as ppool,
            tc.tile_pool(name="upool", bufs=2) as upool,
            tc.tile_pool(name="ixpool", bufs=2) as ixpool,
            tc.tile_pool(name="dstp", bufs=3) as dstp,
        ):
            # ---------------- static tables / params ----------------
            out_v = out_d.ap().rearrange("(j p) s -> p j s", p=128)
            warm = singles.tile([1, 2], f16)
            nc.vector.memset(warm, 0.0)
            warm2 = singles.tile([1, 2], f16)
            nc.scalar.copy(out=warm2, in_=warm)

            # K chunks + blocksum matmuls + M per batch
            kc_sb = [singles.tile([128, 64 * D], f16, name=f"kcs{b}")
                     for b in range(B)]
            M_sb = [singles.tile([D, NB], f16, name=f"Msb{b}")
                    for b in range(B)]
            qT_sb = singles.tile([D, ROWS], f16)
            KQ = 16 * D   # 16 chunks per DMA piece -> 4 pieces per batch

            def load_kc(b, pieces=1):
                n = (64 * D) // pieces
                for q in range(pieces):
                    nc.sync.dma_start(out=kc_sb[b][:, q * n:(q + 1) * n],
                                      in_=kc_d[b].ap()[:, q * n:(q + 1) * n])

            def make_m(b):
                kp = sc_ps.tile([D, NB], f32, name=f"kps{b}", tag="scps")
                for c in range(64):
                    nc.tensor.matmul(kp[:, 2 * c:2 * c + 2],
                                     lhsT=kc_sb[b][:, c * D:(c + 1) * D],
                                     rhs=blkind_sb,
                                     start=True, stop=True)
                ks = small.tile([D, NB], f32, name=f"ksum{b}", tag="ksum")
                nc.scalar.copy(out=ks, in_=kp)
                mp = sc_ps.tile([D, NB], f32, name=f"mps{b}", tag="scps")
                nc.tensor.matmul(mp, lhsT=gT_sb, rhs=ks, start=True, stop=True)
                nc.scalar.copy(out=M_sb[b], in_=mp)

            # critical-path DMA order: kc0, first q chunk, small bundles,
            # kc1, rest of q, index tables
            load_kc(0, pieces=2)
            bun16_sb = singles.tile([128, 2 + 2 * GB], f16)
            nc.sync.dma_start(out=bun16_sb, in_=bun16_d.ap())
            nc.sync.dma_start(out=qT_sb[:, 1024:3072],
                              in_=qT_d.ap()[:, 1024:3072])
            gT_sb = singles.tile([D, D], f32)
            nc.sync.dma_start(out=gT_sb, in_=gT_d.ap())
            zapone_sb = singles.tile([1, 129], f16)
            nc.sync.dma_start(out=zapone_sb, in_=zapone_d.ap())
            blkind_sb = bun16_sb[:, 0:2]
            pf_sb = bun16_sb[:, 2:2 + GB]
            pr_sb = bun16_sb[:, 2 + GB:2 + 2 * GB]
            zap_sb = zapone_sb[:, 0:128]
            one_sb = zapone_sb[:, 128:129]
            make_m(0)
            load_kc(1)
            qranges = [(3072, 5120), (5120, 8192), (12288, 16384),
                       (8192, 12288)]
            for lo, hi in qranges:
                nc.sync.dma_start(out=qT_sb[:, lo:hi],
                                  in_=qT_d.ap()[:, lo:hi])

            buni_sb = singles.tile([128, NTILES_SKIP * SEL], i16)
            nc.sync.dma_start(out=buni_sb, in_=buni_d.ap())
            early_v = buni_sb[:, :].rearrange(
                "p (a b) -> p a b", a=NTILES_SKIP)

            def emit_early_out():
                for b in range(B):
                    jb = b * TPB
                    nc.sync.dma_start(out=out_v[:, jb:jb + NTILES_SKIP, :],
                                      in_=early_v)

            # gate + iota masters per width class (run during the DMA wait)
            gate_cls = {}
            iota_cls = {}
            for Wc in CLASSES:
                gt = singles.tile([128, GB, Wc + 1], f16, name=f"gate{Wc}")
                nc.gpsimd.memset(gt.rearrange("p a b -> p (a b)"), 1.0)
                nc.gpsimd.memset(gt[:, :, 0:1], 0.0)
                gate_cls[Wc] = gt
                it = singles.tile([128, GB * Wc], i16, name=f"iota{Wc}")
                nc.gpsimd.iota(it[:, :], pattern=[[0, GB], [1, Wc]],
                               base=0, channel_multiplier=0)
                iota_cls[Wc] = it

            # ---------------- main loop (software-pipelined) ----------------
            GRP_COL0 = [sum(n for _, n in GRPS[:G]) for G in range(len(GRPS))]

            def extraction_steps(G, scg, vgrp):
                """Deferred per-group extraction, emitted during group G+1's
                tile loop so the in-order DVE/Pool queues never head-of-line
                block on cross-engine dependencies."""
                s0, gb = GRPS[G]
                tiles = TILES[s0:s0 + gb]
                b = tiles[0][0]
                rev = tiles[0][1] > tiles[-1][1]
                Wc = GRP_WMAX[G]
                # plant slice: fwd groups use pf[:gb]; rev use pr tail
                plant = (pr_sb[:, GB - gb:GB] if rev else pf_sb[:, 0:gb])
                state = {}

                def s_tau():
                    tau32 = taupool.tile([128, gb, 1], mybir.dt.float32,
                                      name=f"tau{G}", tag="tau32")
                    nc.vector.tensor_scalar(tau32, vgrp[:, :, 15:16], 1.0,
                                            0.0, op0=mybir.AluOpType.mult,
                                            op1=mybir.AluOpType.add)
                    cpos = cpool.tile([128, gb, Wc + 1], f16, name=f"cp{G}",
                                     tag="cpos")
                    nc.vector.tensor_scalar(
                        cpos[:, :, 0:1], plant.unsqueeze(2), 1.0,
                        0.0, op0=mybir.AluOpType.mult,
                        op1=mybir.AluOpType.add)
                    state["tau32"] = tau32
                    state["cpos"] = cpos

                def s_cpos(lo, hi, dve=False):
                    eng = nc.vector if dve else nc.gpsimd

                    def f():
                        cpos, tau32 = state["cpos"], state["tau32"]
                        for gl in range(lo, hi):
                            eng.tensor_scalar(
                                cpos[:, gl, 1:], scg[:, gl, :],
                                tau32[:, gl, :], 1.0,
                                op0=mybir.AluOpType.is_ge,
                                op1=mybir.AluOpType.mult)
                    return f

                def s_scan():
                    cpos = state["cpos"]
                    P = ppool.tile([128, gb, Wc + 1], f16, name=f"P{G}",
                                  tag="P")
                    nc.vector.tensor_tensor_scan(
                        P.rearrange("p a b -> p (a b)"),
                        gate_cls[Wc][:, 0:gb, :].rearrange("p a b -> p (a b)"),
                        cpos.rearrange("p a b -> p (a b)"),
                        0.0, op0=mybir.AluOpType.mult,
                        op1=mybir.AluOpType.add)
                    state["P"] = P

                def s_u():
                    u = upool.tile([128, gb, Wc + 1], f16, name=f"u{G}",
                                   tag="u")
                    ueng = nc.gpsimd if (not rev and Wc <= 72) else nc.vector
                    ueng.tensor_scalar(
                        u[:, :, :], state["cpos"][:, :, :], BIGC, 0.0,
                        op0=mybir.AluOpType.mult, op1=mybir.AluOpType.add)
                    state["u"] = u

                def s_idx():
                    idx = ixpool.tile([128, gb * Wc], i16, name=f"ix{G}",
                                    tag="idx")
                    nc.vector.tensor_tensor(
                        idx[:, :].rearrange("p (a b) -> p a b", a=gb),
                        state["u"][:, :, 1:], state["P"][:, :, 1:],
                        mybir.AluOpType.add)
                    state["idx"] = idx

                def s_scatter():
                    dst = dstp.tile([128, gb * BINS], i16, name=f"d{G}",
                                    tag="dst")
                    nc.gpsimd.local_scatter(
                        dst[:, :], iota_cls[Wc][:, 0:gb * Wc],
                        state["idx"][:, :],
                        channels=128, num_elems=gb * BINS, num_idxs=gb * Wc)
                    state["dst"] = dst

                def s_out():
                    dview = state["dst"][:, :].rearrange(
                        "p (a b) -> p a b", a=gb)
                    jb = b * TPB + min(i for _, i in tiles)
                    nc.sync.dma_start(out=out_v[:, jb:jb + gb, :],
                                      in_=dview[:, :, 1:17])

                if G == len(GRPS) - 1:
                    state["cpos"] = lcpos
                    return [s_u, s_scan, s_idx, s_scatter, s_out]
                s_tau()
                if gb == GB:
                    return [s_cpos(0, 7), s_cpos(7, gb, dve=rev), None, None,
                            None, None, None, None, s_u, s_scan, s_idx,
                            s_scatter, s_out]
                h = min(4, gb)
                steps = [s_cpos(0, h)]
                if gb > h:
                    steps.append(s_cpos(h, gb, dve=rev))
                steps += [None, s_u, s_scan, s_idx, s_scatter, s_out]
                return steps

            pending = []
            for G, (s0, gb) in enumerate(GRPS):
                tiles = TILES[s0:s0 + gb]
                Wc = GRP_WMAX[G]
                scg = scgp.tile([128, gb, Wc], f16, name=f"scg{G}", tag="scg")
                vgrp = vpool.tile([128, gb, SEL], f16, name=f"v{G}", tag="v")
                lastg = (G == len(GRPS) - 1)
                if lastg:
                    lcpos = cpool.tile([128, gb, Wc + 1], f16,
                                       name=f"lcp{G}", tag="cpos")
                    nc.vector.tensor_scalar(
                        lcpos[:, :, 0:1], pr_sb[:, GB - gb:GB].unsqueeze(2),
                        1.0, 0.0, op0=mybir.AluOpType.mult,
                        op1=mybir.AluOpType.add)
                for gl, (bb, i) in enumerate(tiles):
                    W = 2 * i + 2
                    colbase = bb * T + i * 128
                    ps = sc_ps.tile([128, Wc], f32, name=f"ps{G}_{gl}",
                                    tag="scps")
                    nc.tensor.matmul(ps, lhsT=qT_sb[:, colbase:colbase + 128],
                                     rhs=M_sb[bb][:, 0:Wc],
                                     start=True, stop=False)
                    nc.tensor.matmul(ps[:, 2 * i + 1:2 * i + 2],
                                     lhsT=zap_sb, rhs=one_sb,
                                     start=False, stop=True)
                    nc.scalar.copy(out=scg[:, gl, :], in_=ps)
                    sc2 = small.tile([128, W], f16, name=f"sc2_{G}_{gl}",
                                     tag="sc2")
                    va = vgrp[:, gl, 0:8]
                    vb = vgrp[:, gl, 8:16]
                    nc.vector.max(out=va, in_=scg[:, gl, 0:W])
                    nc.vector.match_replace(out=sc2, in_to_replace=va,
                                            in_values=scg[:, gl, 0:W],
                                            imm_value=ZAPV)
                    nc.vector.max(out=vb, in_=sc2)
                    if lastg:
                        ltau = taupool.tile([128, 1], mybir.dt.float32,
                                            name=f"lt{G}_{gl}", tag="ltau")
                        nc.vector.tensor_scalar(
                            ltau, vgrp[:, gl, 15:16], 1.0, 0.0,
                            op0=mybir.AluOpType.mult,
                            op1=mybir.AluOpType.add)
                        nc.vector.tensor_scalar(
                            lcpos[:, gl, 1:], scg[:, gl, :], ltau, 1.0,
                            op0=mybir.AluOpType.is_ge,
                            op1=mybir.AluOpType.mult)
                    if pending:
                        s = pending.pop(0)
                        if s is not None:
                            s()
                for s in pending:       # drain any leftovers at group end
                    if s is not None:
                        s()
                if G == 1:
                    make_m(1)
                    emit_early_out()
                pending = extraction_steps(G, scg, vgrp)
            for s in pending:
                if s is not None:
                    s()

    nc.compile()
    return nc


def _shard_inputs(Q, K, Wq, Wk):
    early, pf, pr, zap, one, blkind = _static_tables()
    in_maps = []
    for h in range(H):
        qT = np.ascontiguousarray(
            Q[:, :, GROUPS * h, :].reshape(ROWS, D).T).astype(np.float16)
        kcs = {}
        for b in range(B):
            kb = K[b, :, h, :].reshape(64, 128, D).transpose(1, 0, 2)
            kcs[f"kc{b}"] = np.ascontiguousarray(
                kb.reshape(128, 64 * D)).astype(np.float16)
        G = (Wq[h].astype(np.float64)
             @ Wk[h].astype(np.float64).T / BS).astype(np.float32)
        bun16 = np.hstack([blkind, pf, pr]).astype(np.float16)
        zapone = np.hstack([zap, one]).astype(np.float16)
        buni = early.astype(np.int16)
        in_maps.append({
            "qT": qT, **kcs,
            "gT": np.ascontiguousarray(G.T),
            "bun16": bun16, "zapone": zapone, "buni": buni,
        })
    return in_maps


def kernel(Q, K, Wq, Wk, logit_scale=None, block_size=64, selected_blocks=16,
           groups=4, **_unused):
    assert int(block_size) == BS and int(selected_blocks) == SEL
    assert int(groups) == GROUPS
    Q = np.asarray(Q, np.float32)
    K = np.asarray(K, np.float32)
    Wq = np.asarray(Wq, np.float32)
    Wk = np.asarray(Wk, np.float32)
    # exp(logit_scale) > 0 scales scores per-head only -> ranking unchanged.

    if "nc" not in _CACHE:
        _CACHE["nc"] = build_program()
    nc = _CACHE["nc"]

    in_maps = _shard_inputs(Q, K, Wq, Wk)
    res = run_bass_kernel_spmd(nc, in_maps, core_ids=list(range(H)))
    outs = [res.results[h]["out"] for h in range(H)]          # [ROWS, SEL] i16
    out = np.stack(outs, axis=1).reshape(B, T, H, SEL).astype(np.int32)
    # union-with-locals clamp: out[..., 15] = min(out[..., 15], t_blk - 1)
    # (early rows t < 1024 come from the static table and are left as-is)
    tbm1 = np.maximum(np.arange(T) // BS - 1, 0).astype(np.int32)
    out[:, 1024:, :, 15] = np.minimum(out[:, 1024:, :, 15],
                                      tbm1[1024:, None])
    return out


if __name__ == "__main__":
    rng = np.random.default_rng(0)
    Q = rng.standard_normal((B, T, HQ, D)).astype(np.float32)
    K = rng.standard_normal((B, T, H, D)).astype(np.float32)
    Wq = (rng.standard_normal((H, D, DR)) * 0.02).astype(np.float32)
    Wk = (rng.standard_normal((H, D, DR)) * 0.02).astype(np.float32)
    out = kernel(Q=Q, K=K, Wq=Wq, Wk=Wk)
    print("kernel ran:", out.shape, out.dtype)


# revision 6
# speedup vs baseline: 1.0883x; 1.0032x over previous
"""Trainium2 Bass kernel v2 for nn_MiniDSARouter (topk block routing).

Shapes: B=2, T=8192, HQ=32, H=8, D=64, DR=16, block_size=64,
selected_blocks=16, groups=4, ADD_LOCAL=1. One KV head per core.

Semantics (same reduction as baseline, verified vs reference):
  out[b,t,h,:] = sorted_asc(top16_idx(scores[b,t,h,:]))
  with out[15] := min(out[15], t_blk-1)
where scores = q^T (Wq Wk^T/64) ksum^T with causal block mask, and
rows with t_blk <= 15 are a static function of t.

v2 pipeline per 128-row tile (vs baseline's 6 DVE passes + Act copy):
  PE:   scores = qT^T @ M in ONE fp16 matmul (M = G @ blocksum(K),
        G = Wq Wk^T/64 precomputed on host), plus fp16 "zap" matmul
        masking block 2i+1 for rows p<64.
  Act:  copy PSUM fp32 -> SBUF fp16 group buffer.
  DVE:  Max8 / MatchReplace / Max8 -> top-16 values, tau = v[15].
Then per group of 14 tiles (batched ops):
  cpos = (sc >= tau) via one TT is_ge (Pool), with a planted column per
  tile carrying 64*gl; one gated segmented scan (Pool) turns marks into
  per-column rank P + 64*gl; one scalar_tensor_tensor (DVE, 4x fp16)
  maps marked columns to unique bins P+64*gl and unmarked to negative;
  one batched gpsimd local_scatter writes column index j into bin
  rank-1 -- dst bins [64gl+1 .. 64gl+16] ARE the sorted top-16 indices.
  One TT min clamps slot 15 with t_blk-1. DMA out as int16.

All DRAM I/O is fp16/int16 (half the baseline's DMA traffic).
"""

import numpy as np

import concourse.bass as bass
import concourse.mybir as mybir
import concourse.tile as tile
from concourse import bacc
from concourse.bass_utils import run_bass_kernel_spmd

B, T, HQ, H, D, DR = 2, 8192, 32, 8, 64, 16
BS = 64
NB = T // BS               # 128 blocks per batch
SEL = 16
GROUPS = 4
ROWS = B * T               # 16384 rows per core
NTILES_SKIP = 8            # per-batch tiles 0..7 (t < 1024) are static
TPB = T // 128             # 64 row-tiles per batch
NGT = TPB - NTILES_SKIP    # 56 computed tiles per batch
NG = B * NGT               # 112 computed tiles per core
GB = 14                    # tiles per group
NGRP = NG // GB            # 8 groups
BINS = 64                  # scatter bins per tile
BIGC = 1024.0              # unmarked-to-negative shift
ZAPV = -60000.0

_CACHE = {}


def _tiles():
    """b0 ascending i, then b1 descending i (drain on cheap tiles)."""
    return ([(0, NTILES_SKIP + k) for k in range(NGT)]
            + [(1, TPB - 1 - k) for k in range(NGT)])


TILES = _tiles()
# groups: 7x14 tiles + 2x7 (short tail groups drain the pipeline fast)
GRPS = [(14 * k, 14) for k in range(7)] + [(98, 7), (105, 7)]
GRP_WMAX = [max(2 * i + 2 for _, i in TILES[s:s + n]) for s, n in GRPS]
CLASSES = sorted(set(GRP_WMAX))                      # [44, 72, 100, 128]


def _static_tables():
    # early rows: t_blk <= 15 -> sorted([0..15] + [t_blk, max(t_blk-1,0)])[:16]
    early = np.empty((128, NTILES_SKIP, SEL), np.int16)
    for t in range(NTILES_SKIP * 128):
        tb = t // BS
        s = sorted(list(range(16)) + [tb, max(tb - 1, 0)])
        early[t % 128, t // 128] = s[:SEL]
    early = early.reshape(128, NTILES_SKIP * SEL)

    # plants: per-tile scan seed = 64*outpos - BIGC, so the stt's
    # (cpos*BIGC + P') lands marked cols at bin 64*outpos + rank and
    # unmarked cols strictly negative. fwd for b0 groups, rev for b1.
    pf = np.tile((np.arange(GB) * BINS - BIGC).astype(np.float16), (128, 1))
    pr = pf[:, ::-1].copy()
    zap = np.zeros((1, 128), np.float16)
    zap[0, :64] = ZAPV
    one = np.ones((1, 1), np.float16)
    blkind = np.zeros((128, 2), np.float16)
    blkind[:64, 0] = 1.0
    blkind[64:, 1] = 1.0
    return early, pf, pr, zap, one, blkind


def build_program():
    f32 = mybir.dt.float32
    f16 = mybir.dt.float16
    i16 = mybir.dt.int16
    nc = bacc.Bacc("TRN2", target_bir_lowering=False, debug=False)

    qT_d = nc.dram_tensor("qT", [D, ROWS], f16, kind="ExternalInput")
    kc_d = [nc.dram_tensor(f"kc{b}", [128, 64 * D], f16, kind="ExternalInput")
            for b in range(B)]
    gT_d = nc.dram_tensor("gT", [D, D], f32, kind="ExternalInput")
    bun16_d = nc.dram_tensor("bun16", [128, 2 + 2 * GB], f16,
                             kind="ExternalInput")
    zapone_d = nc.dram_tensor("zapone", [1, 129], f16, kind="ExternalInput")
    buni_d = nc.dram_tensor("buni", [128, NTILES_SKIP * SEL], i16,
                            kind="ExternalInput")
    out_d = nc.dram_tensor("out", [ROWS, SEL], i16, kind="ExternalOutput")

    with tile.TileContext(nc) as tc:
        with (
            tc.tile_pool(name="singles", bufs=1) as singles,
            tc.tile_pool(name="qchunk", bufs=1) as qpool,
            tc.tile_pool(name="sc_ps", bufs=8, space="PSUM") as sc_ps,
            tc.tile_pool(name="scg", bufs=4) as scgp,
            tc.tile_pool(name="small", bufs=8) as small,
            tc.tile_pool(name="vpool", bufs=3) as vpool,
            tc.tile_pool(name="taupool", bufs=3) as taupool,
            tc.tile_pool(name="cpool", bufs=2) as cpool,
            tc.tile_pool(name="ppool", bufs=2) as ppool,
            tc.tile_pool(name="upool", bufs=2) as upool,
            tc.tile_pool(name="ixpool", bufs=2) as ixpool,
            tc.tile_pool(name="dstp", bufs=3) as dstp,
        ):
            # ---------------- static tables / params ----------------
            out_v = out_d.ap().rearrange("(j p) s -> p j s", p=128)
            warm = singles.tile([1, 2], f16)
            nc.vector.memset(warm, 0.0)
            warm2 = singles.tile([1, 2], f16)
            nc.scalar.copy(out=warm2, in_=warm)

            # K chunks + blocksum matmuls + M per batch
            kc_sb = [singles.tile([128, 64 * D], f16, name=f"kcs{b}")
                     for b in range(B)]
            M_sb = [singles.tile([D, NB], f16, name=f"Msb{b}")
                    for b in range(B)]
            qT_sb = singles.tile([D, ROWS], f16)
            KQ = 16 * D   # 16 chunks per DMA piece -> 4 pieces per batch

            def load_kc(b, pieces=1):
                n = (64 * D) // pieces
                for q in range(pieces):
                    nc.sync.dma_start(out=kc_sb[b][:, q * n:(q + 1) * n],
                                      in_=kc_d[b].ap()[:, q * n:(q + 1) * n])

            def make_m(b):
                kp = sc_ps.tile([D, NB], f32, name=f"kps{b}", tag="scps")
                for c in range(64):
                    nc.tensor.matmul(kp[:, 2 * c:2 * c + 2],
                                     lhsT=kc_sb[b][:, c * D:(c + 1) * D],
                                     rhs=blkind_sb,
                                     start=True, stop=True)
                ks = small.tile([D, NB], f32, name=f"ksum{b}", tag="ksum")
                nc.scalar.copy(out=ks, in_=kp)
                mp = sc_ps.tile([D, NB], f32, name=f"mps{b}", tag="scps")
                nc.tensor.matmul(mp, lhsT=gT_sb, rhs=ks, start=True, stop=True)
                nc.scalar.copy(out=M_sb[b], in_=mp)

            # critical-path DMA order: kc0, first q chunk, small bundles,
            # kc1, rest of q, index tables
            load_kc(0, pieces=2)
            bun16_sb = singles.tile([128, 2 + 2 * GB], f16)
            nc.sync.dma_start(out=bun16_sb, in_=bun16_d.ap())
            nc.sync.dma_start(out=qT_sb[:, 1024:3072],
                              in_=qT_d.ap()[:, 1024:3072])
            gT_sb = singles.tile([D, D], f32)
            nc.sync.dma_start(out=gT_sb, in_=gT_d.ap())
            zapone_sb = singles.tile([1, 129], f16)
            nc.sync.dma_start(out=zapone_sb, in_=zapone_d.ap())
            blkind_sb = bun16_sb[:, 0:2]
            pf_sb = bun16_sb[:, 2:2 + GB]
            pr_sb = bun16_sb[:, 2 + GB:2 + 2 * GB]
            zap_sb = zapone_sb[:, 0:128]
            one_sb = zapone_sb[:, 128:129]
            make_m(0)
            load_kc(1)
            qranges = [(3072, 5120), (5120, 8192), (12288, 16384),
                       (8192, 12288)]
            for lo, hi in qranges:
                nc.sync.dma_start(out=qT_sb[:, lo:hi],
                                  in_=qT_d.ap()[:, lo:hi])

            buni_sb = singles.tile([128, NTILES_SKIP * SEL], i16)
            nc.sync.dma_start(out=buni_sb, in_=buni_d.ap())
            early_v = buni_sb[:, :].rearrange(
                "p (a b) -> p a b", a=NTILES_SKIP)

            def emit_early_out():
                for b in range(B):
                    jb = b * TPB
                    nc.sync.dma_start(out=out_v[:, jb:jb + NTILES_SKIP, :],
                                      in_=early_v)

            # gate + iota masters per width class (run during the DMA wait)
            gate_cls = {}
            iota_cls = {}
            for Wc in CLASSES:
                gt = singles.tile([128, GB, Wc + 1], f16, name=f"gate{Wc}")
                nc.gpsimd.memset(gt.rearrange("p a b -> p (a b)"), 1.0)
                nc.gpsimd.memset(gt[:, :, 0:1], 0.0)
                gate_cls[Wc] = gt
                it = singles.tile([128, GB * Wc], i16, name=f"iota{Wc}")
                nc.gpsimd.iota(it[:, :], pattern=[[0, GB], [1, Wc]],
                               base=0, channel_multiplier=0)
                iota_cls[Wc] = it

            # ---------------- main loop (software-pipelined) ----------------
            GRP_COL0 = [sum(n for _, n in GRPS[:G]) for G in range(len(GRPS))]

            def extraction_steps(G, scg, vgrp):
                """Deferred per-group extraction, emitted during group G+1's
                tile loop so the in-order DVE/Pool queues never head-of-line
                block on cross-engine dependencies."""
                s0, gb = GRPS[G]
                tiles = TILES[s0:s0 + gb]
                b = tiles[0][0]
                rev = tiles[0][1] > tiles[-1][1]
                Wc = GRP_WMAX[G]
                # plant slice: fwd groups use pf[:gb]; rev use pr tail
                plant = (pr_sb[:, GB - gb:GB] if rev else pf_sb[:, 0:gb])
                state = {}

                def s_tau():
                    tau32 = taupool.tile([128, gb, 1], mybir.dt.float32,
                                      name=f"tau{G}", tag="tau32")
                    nc.vector.tensor_scalar(tau32, vgrp[:, :, 15:16], 1.0,
                                            0.0, op0=mybir.AluOpType.mult,
                                            op1=mybir.AluOpType.add)
                    cpos = cpool.tile([128, gb, Wc + 1], f16, name=f"cp{G}",
                                     tag="cpos")
                    nc.vector.tensor_scalar(
                        cpos[:, :, 0:1], plant.unsqueeze(2), 1.0,
                        0.0, op0=mybir.AluOpType.mult,
                        op1=mybir.AluOpType.add)
                    state["tau32"] = tau32
                    state["cpos"] = cpos

                def s_cpos(lo, hi, dve=False):
                    eng = nc.vector if dve else nc.gpsimd

                    def f():
                        cpos, tau32 = state["cpos"], state["tau32"]
                        for gl in range(lo, hi):
                            eng.tensor_scalar(
                                cpos[:, gl, 1:], scg[:, gl, :],
                                tau32[:, gl, :], 1.0,
                                op0=mybir.AluOpType.is_ge,
                                op1=mybir.AluOpType.mult)
                    return f

                def s_scan():
                    cpos = state["cpos"]
                    P = ppool.tile([128, gb, Wc + 1], f16, name=f"P{G}",
                                  tag="P")
                    nc.vector.tensor_tensor_scan(
                        P.rearrange("p a b -> p (a b)"),
                        gate_cls[Wc][:, 0:gb, :].rearrange("p a b -> p (a b)"),
                        cpos.rearrange("p a b -> p (a b)"),
                        0.0, op0=mybir.AluOpType.mult,
                        op1=mybir.AluOpType.add)
                    state["P"] = P

                def s_u():
                    u = upool.tile([128, gb, Wc + 1], f16, name=f"u{G}",
                                   tag="u")
                    ueng = nc.gpsimd if (not rev and Wc <= 100) else nc.vector
                    ueng.tensor_scalar(
                        u[:, :, :], state["cpos"][:, :, :], BIGC, 0.0,
                        op0=mybir.AluOpType.mult, op1=mybir.AluOpType.add)
                    state["u"] = u

                def s_idx():
                    idx = ixpool.tile([128, gb * Wc], i16, name=f"ix{G}",
                                    tag="idx")
                    nc.vector.tensor_tensor(
                        idx[:, :].rearrange("p (a b) -> p a b", a=gb),
                        state["u"][:, :, 1:], state["P"][:, :, 1:],
                        mybir.AluOpType.add)
                    state["idx"] = idx

                def s_scatter():
                    dst = dstp.tile([128, gb * BINS], i16, name=f"d{G}",
                                    tag="dst")
                    nc.gpsimd.local_scatter(
                        dst[:, :], iota_cls[Wc][:, 0:gb * Wc],
                        state["idx"][:, :],
                        channels=128, num_elems=gb * BINS, num_idxs=gb * Wc)
                    state["dst"] = dst

                def s_out():
                    dview = state["dst"][:, :].rearrange(
                        "p (a b) -> p a b", a=gb)
                    jb = b * TPB + min(i for _, i in tiles)
                    nc.sync.dma_start(out=out_v[:, jb:jb + gb, :],
                                      in_=dview[:, :, 1:17])

                if G == len(GRPS) - 1:
                    state["cpos"] = lcpos
                    return [s_u, s_scan, s_idx, s_scatter, s_out]
                s_tau()
                if gb == GB:
                    return [s_cpos(0, 7), s_cpos(7, gb, dve=rev), None, None,
                            None, None, None, None, s_u, s_scan, s_idx,
                            s_scatter, s_out]
                h = min(4, gb)
                steps = [s_cpos(0, h)]
                if gb > h:
                    steps.append(s_cpos(h, gb, dve=rev))
                steps += [None, s_u, s_scan, s_idx, s_scatter, s_out]
                return steps

            pending = []
            for G, (s0, gb) in enumerate(GRPS):
                tiles = TILES[s0:s0 + gb]
                Wc = GRP_WMAX[G]
                scg = scgp.tile([128, gb, Wc], f16, name=f"scg{G}", tag="scg")
                vgrp = vpool.tile([128, gb, SEL], f16, name=f"v{G}", tag="v")
                lastg = (G == len(GRPS) - 1)
                if lastg:
                    lcpos = cpool.tile([128, gb, Wc + 1], f16,
                                       name=f"lcp{G}", tag="cpos")
                    nc.vector.tensor_scalar(
                        lcpos[:, :, 0:1], pr_sb[:, GB - gb:GB].unsqueeze(2),
                        1.0, 0.0, op0=mybir.AluOpType.mult,
                        op1=mybir.AluOpType.add)
                for gl, (bb, i) in enumerate(tiles):
                    W = 2 * i + 2
                    colbase = bb * T + i * 128
                    ps = sc_ps.tile([128, Wc], f32, name=f"ps{G}_{gl}",
                                    tag="scps")
                    nc.tensor.matmul(ps, lhsT=qT_sb[:, colbase:colbase + 128],
                                     rhs=M_sb[bb][:, 0:Wc],
                                     start=True, stop=False)
                    nc.tensor.matmul(ps[:, 2 * i + 1:2 * i + 2],
                                     lhsT=zap_sb, rhs=one_sb,
                                     start=False, stop=True)
                    nc.scalar.copy(out=scg[:, gl, :], in_=ps)
                    sc2 = small.tile([128, W], f16, name=f"sc2_{G}_{gl}",
                                     tag="sc2")
                    va = vgrp[:, gl, 0:8]
                    vb = vgrp[:, gl, 8:16]
                    nc.vector.max(out=va, in_=scg[:, gl, 0:W])
                    nc.vector.match_replace(out=sc2, in_to_replace=va,
                                            in_values=scg[:, gl, 0:W],
                                            imm_value=ZAPV)
                    nc.vector.max(out=vb, in_=sc2)
                    if lastg:
                        ltau = taupool.tile([128, 1], mybir.dt.float32,
                                            name=f"lt{G}_{gl}", tag="ltau")
                        nc.vector.tensor_scalar(
                            ltau, vgrp[:, gl, 15:16], 1.0, 0.0,
                            op0=mybir.AluOpType.mult,
                            op1=mybir.AluOpType.add)
                        nc.vector.tensor_scalar(
                            lcpos[:, gl, 1:], scg[:, gl, :], ltau, 1.0,
                            op0=mybir.AluOpType.is_ge,
                            op1=mybir.AluOpType.mult)
                    if pending:
                        s = pending.pop(0)
                        if s is not None:
                            s()
                for s in pending:       # drain any leftovers at group end
                    if s is not None:
                        s()
                if G == 1:
                    make_m(1)
                    emit_early_out()
                pending = extraction_steps(G, scg, vgrp)
            for s in pending:
                if s is not None:
                    s()

    nc.compile()
    return nc


def _shard_inputs(Q, K, Wq, Wk):
    early, pf, pr, zap, one, blkind = _static_tables()
    in_maps = []
    for h in range(H):
        qT = np.ascontiguousarray(
            Q[:, :, GROUPS * h, :].reshape(ROWS, D).T).astype(np.float16)
        kcs = {}
        for b in range(B):
            kb = K[b, :, h, :].reshape(64, 128, D).transpose(1, 0, 2)
            kcs[f"kc{b}"] = np.ascontiguousarray(
                kb.reshape(128, 64 * D)).astype(np.float16)
        G = (Wq[h].astype(np.float64)
             @ Wk[h].astype(np.float64).T / BS).astype(np.float32)
        bun16 = np.hstack([blkind, pf, pr]).astype(np.float16)
        zapone = np.hstack([zap, one]).astype(np.float16)
        buni = early.astype(np.int16)
        in_maps.append({
            "qT": qT, **kcs,
            "gT": np.ascontiguousarray(G.T),
            "bun16": bun16, "zapone": zapone, "buni": buni,
        })
    return in_maps


def kernel(Q, K, Wq, Wk, logit_scale=None, block_size=64, selected_blocks=16,
           groups=4, **_unused):
    assert int(block_size) == BS and int(selected_blocks) == SEL
    assert int(groups) == GROUPS
    Q = np.asarray(Q, np.float32)
    K = np.asarray(K, np.float32)
    Wq = np.asarray(Wq, np.float32)
    Wk = np.asarray(Wk, np.float32)
    # exp(logit_scale) > 0 scales scores per-head only -> ranking unchanged.

    if "nc" not in _CACHE:
        _CACHE["nc"] = build_program()
    nc = _CACHE["nc"]

    in_maps = _shard_inputs(Q, K, Wq, Wk)
    res = run_bass_kernel_spmd(nc, in_maps, core_ids=list(range(H)))
    outs = [res.results[h]["out"] for h in range(H)]          # [ROWS, SEL] i16
    out = np.stack(outs, axis=1).reshape(B, T, H, SEL).astype(np.int32)
    # union-with-locals clamp: out[..., 15] = min(out[..., 15], t_blk - 1)
    # (early rows t < 1024 come from the static table and are left as-is)
    tbm1 = np.maximum(np.arange(T) // BS - 1, 0).astype(np.int32)
    out[:, 1024:, :, 15] = np.minimum(out[:, 1024:, :, 15],
                                      tbm1[1024:, None])
    return out


if __name__ == "__main__":
    rng = np.random.default_rng(0)
    Q = rng.standard_normal((B, T, HQ, D)).astype(np.float32)
    K = rng.standard_normal((B, T, H, D)).astype(np.float32)
    Wq = (rng.standard_normal((H, D, DR)) * 0.02).astype(np.float32)
    Wk = (rng.standard_normal((H, D, DR)) * 0.02).astype(np.float32)
    out = kernel(Q=Q, K=K, Wq=Wq, Wk=Wk)
    print("kernel ran:", out.shape, out.dtype)


# revision 8
# speedup vs baseline: 1.0928x; 1.0042x over previous
"""Trainium2 Bass kernel v2 for nn_MiniDSARouter (topk block routing).

Shapes: B=2, T=8192, HQ=32, H=8, D=64, DR=16, block_size=64,
selected_blocks=16, groups=4, ADD_LOCAL=1. One KV head per core.

Semantics (same reduction as baseline, verified vs reference):
  out[b,t,h,:] = sorted_asc(top16_idx(scores[b,t,h,:]))
  with out[15] := min(out[15], t_blk-1)
where scores = q^T (Wq Wk^T/64) ksum^T with causal block mask, and
rows with t_blk <= 15 are a static function of t.

v2 pipeline per 128-row tile (vs baseline's 6 DVE passes + Act copy):
  PE:   scores = qT^T @ M in ONE fp16 matmul (M = G @ blocksum(K),
        G = Wq Wk^T/64 precomputed on host), plus fp16 "zap" matmul
        masking block 2i+1 for rows p<64.
  Act:  copy PSUM fp32 -> SBUF fp16 group buffer.
  DVE:  Max8 / MatchReplace / Max8 -> top-16 values, tau = v[15].
Then per group of 14 tiles (batched ops):
  cpos = (sc >= tau) via one TT is_ge (Pool), with a planted column per
  tile carrying 64*gl; one gated segmented scan (Pool) turns marks into
  per-column rank P + 64*gl; one scalar_tensor_tensor (DVE, 4x fp16)
  maps marked columns to unique bins P+64*gl and unmarked to negative;
  one batched gpsimd local_scatter writes column index j into bin
  rank-1 -- dst bins [64gl+1 .. 64gl+16] ARE the sorted top-16 indices.
  One TT min clamps slot 15 with t_blk-1. DMA out as int16.

All DRAM I/O is fp16/int16 (half the baseline's DMA traffic).
"""

import numpy as np

import concourse.bass as bass
import concourse.mybir as mybir
import concourse.tile as tile
from concourse import bacc
from concourse.bass_utils import run_bass_kernel_spmd

B, T, HQ, H, D, DR = 2, 8192, 32, 8, 64, 16
BS = 64
NB = T // BS               # 128 blocks per batch
SEL = 16
GROUPS = 4
ROWS = B * T               # 16384 rows per core
NTILES_SKIP = 8            # per-batch tiles 0..7 (t < 1024) are static
TPB = T // 128             # 64 row-tiles per batch
NGT = TPB - NTILES_SKIP    # 56 computed tiles per batch
NG = B * NGT               # 112 computed tiles per core
GB = 14                    # tiles per group
NGRP = NG // GB            # 8 groups
BINS = 64                  # scatter bins per tile
BIGC = 1024.0              # unmarked-to-negative shift
ZAPV = -60000.0

_CACHE = {}


def _tiles():
    """b0 ascending i, then b1 descending i (drain on cheap tiles)."""
    return ([(0, NTILES_SKIP + k) for k in range(NGT)]
            + [(1, TPB - 1 - k) for k in range(NGT)])


TILES = _tiles()
# groups: 7x14 tiles + 2x7 (short tail groups drain the pipeline fast)
GRPS = [(14 * k, 14) for k in range(7)] + [(98, 7), (105, 7)]
GRP_WMAX = [max(2 * i + 2 for _, i in TILES[s:s + n]) for s, n in GRPS]
CLASSES = sorted(set(GRP_WMAX))                      # [44, 72, 100, 128]


def _static_tables():
    # early rows: t_blk <= 15 -> sorted([0..15] + [t_blk, max(t_blk-1,0)])[:16]
    early = np.empty((128, NTILES_SKIP, SEL), np.int16)
    for t in range(NTILES_SKIP * 128):
        tb = t // BS
        s = sorted(list(range(16)) + [tb, max(tb - 1, 0)])
        early[t % 128, t // 128] = s[:SEL]
    early = early.reshape(128, NTILES_SKIP * SEL)

    # plants: per-tile scan seed = 64*outpos - BIGC, so the stt's
    # (cpos*BIGC + P') lands marked cols at bin 64*outpos + rank and
    # unmarked cols strictly negative. fwd for b0 groups, rev for b1.
    pf = np.tile((np.arange(GB) * BINS - BIGC).astype(np.float16), (128, 1))
    pr = pf[:, ::-1].copy()
    zap = np.zeros((1, 128), np.float16)
    zap[0, :64] = ZAPV
    one = np.ones((1, 1), np.float16)
    blkind = np.zeros((128, 2), np.float16)
    blkind[:64, 0] = 1.0
    blkind[64:, 1] = 1.0
    return early, pf, pr, zap, one, blkind


def build_program():
    f32 = mybir.dt.float32
    f16 = mybir.dt.float16
    i16 = mybir.dt.int16
    nc = bacc.Bacc("TRN2", target_bir_lowering=False, debug=False)

    qT_d = nc.dram_tensor("qT", [D, ROWS], f16, kind="ExternalInput")
    kc_d = [nc.dram_tensor(f"kc{b}", [128, 64 * D], f16, kind="ExternalInput")
            for b in range(B)]
    gT_d = nc.dram_tensor("gT", [D, D], f32, kind="ExternalInput")
    bun16_d = nc.dram_tensor("bun16", [128, 2 + 2 * GB], f16,
                             kind="ExternalInput")
    zapone_d = nc.dram_tensor("zapone", [1, 129], f16, kind="ExternalInput")
    buni_d = nc.dram_tensor("buni", [128, NTILES_SKIP * SEL], i16,
                            kind="ExternalInput")
    out_d = nc.dram_tensor("out", [ROWS, SEL], i16, kind="ExternalOutput")

    with tile.TileContext(nc) as tc:
        with (
            tc.tile_pool(name="singles", bufs=1) as singles,
            tc.tile_pool(name="qchunk", bufs=1) as qpool,
            tc.tile_pool(name="sc_ps", bufs=8, space="PSUM") as sc_ps,
            tc.tile_pool(name="scg", bufs=4) as scgp,
            tc.tile_pool(name="small", bufs=8) as small,
            tc.tile_pool(name="vpool", bufs=3) as vpool,
            tc.tile_pool(name="taupool", bufs=3) as taupool,
            tc.tile_pool(name="cpool", bufs=2) as cpool,
            tc.tile_pool(name="ppool", bufs=2) as ppool,
            tc.tile_pool(name="upool", bufs=2) as upool,
            tc.tile_pool(name="ixpool", bufs=2) as ixpool,
            tc.tile_pool(name="dstp", bufs=3) as dstp,
        ):
            # ---------------- static tables / params ----------------
            out_v = out_d.ap().rearrange("(j p) s -> p j s", p=128)
            warm = singles.tile([1, 2], f16)
            nc.vector.memset(warm, 0.0)
            warm2 = singles.tile([1, 2], f16)
            nc.scalar.copy(out=warm2, in_=warm)

            # K chunks + blocksum matmuls + M per batch
            kc_sb = [singles.tile([128, 64 * D], f16, name=f"kcs{b}")
                     for b in range(B)]
            M_sb = [singles.tile([D, NB], f16, name=f"Msb{b}")
                    for b in range(B)]
            qT_sb = singles.tile([D, ROWS], f16)
            KQ = 16 * D   # 16 chunks per DMA piece -> 4 pieces per batch

            def load_kc(b, pieces=1):
                n = (64 * D) // pieces
                for q in range(pieces):
                    nc.sync.dma_start(out=kc_sb[b][:, q * n:(q + 1) * n],
                                      in_=kc_d[b].ap()[:, q * n:(q + 1) * n])

            def make_m(b):
                kp = sc_ps.tile([D, NB], f32, name=f"kps{b}", tag="scps")
                for c in range(64):
                    nc.tensor.matmul(kp[:, 2 * c:2 * c + 2],
                                     lhsT=kc_sb[b][:, c * D:(c + 1) * D],
                                     rhs=blkind_sb,
                                     start=True, stop=True)
                ks = small.tile([D, NB], f32, name=f"ksum{b}", tag="ksum")
                nc.scalar.copy(out=ks, in_=kp)
                mp = sc_ps.tile([D, NB], f32, name=f"mps{b}", tag="scps")
                nc.tensor.matmul(mp, lhsT=gT_sb, rhs=ks, start=True, stop=True)
                nc.scalar.copy(out=M_sb[b], in_=mp)

            # critical-path DMA order: kc0, first q chunk, small bundles,
            # kc1, rest of q, index tables
            load_kc(0, pieces=2)
            bun16_sb = singles.tile([128, 2 + 2 * GB], f16)
            nc.sync.dma_start(out=bun16_sb, in_=bun16_d.ap())
            nc.sync.dma_start(out=qT_sb[:, 1024:3072],
                              in_=qT_d.ap()[:, 1024:3072])
            gT_sb = singles.tile([D, D], f32)
            nc.sync.dma_start(out=gT_sb, in_=gT_d.ap())
            zapone_sb = singles.tile([1, 129], f16)
            nc.sync.dma_start(out=zapone_sb, in_=zapone_d.ap())
            blkind_sb = bun16_sb[:, 0:2]
            pf_sb = bun16_sb[:, 2:2 + GB]
            pr_sb = bun16_sb[:, 2 + GB:2 + 2 * GB]
            zap_sb = zapone_sb[:, 0:128]
            one_sb = zapone_sb[:, 128:129]
            make_m(0)
            load_kc(1)
            qranges = [(3072, 5120), (5120, 8192), (12288, 16384),
                       (8192, 12288)]
            for lo, hi in qranges:
                nc.sync.dma_start(out=qT_sb[:, lo:hi],
                                  in_=qT_d.ap()[:, lo:hi])

            buni_sb = singles.tile([128, NTILES_SKIP * SEL], i16)
            nc.sync.dma_start(out=buni_sb, in_=buni_d.ap())
            early_v = buni_sb[:, :].rearrange(
                "p (a b) -> p a b", a=NTILES_SKIP)

            def emit_early_out():
                for b in range(B):
                    jb = b * TPB
                    nc.sync.dma_start(out=out_v[:, jb:jb + NTILES_SKIP, :],
                                      in_=early_v)

            # gate + iota masters per width class (run during the DMA wait)
            gate_cls = {}
            iota_cls = {}
            for Wc in CLASSES:
                gt = singles.tile([128, GB, Wc + 1], f16, name=f"gate{Wc}")
                nc.vector.memset(gt.rearrange("p a b -> p (a b)"), 1.0)
                nc.vector.memset(gt[:, :, 0:1], 0.0)
                gate_cls[Wc] = gt
                it = singles.tile([128, GB * Wc], i16, name=f"iota{Wc}")
                nc.gpsimd.iota(it[:, :], pattern=[[0, GB], [1, Wc]],
                               base=0, channel_multiplier=0)
                iota_cls[Wc] = it

            # ---------------- main loop (software-pipelined) ----------------
            GRP_COL0 = [sum(n for _, n in GRPS[:G]) for G in range(len(GRPS))]

            def extraction_steps(G, scg, vgrp):
                """Deferred per-group extraction, emitted during group G+1's
                tile loop so the in-order DVE/Pool queues never head-of-line
                block on cross-engine dependencies."""
                s0, gb = GRPS[G]
                tiles = TILES[s0:s0 + gb]
                b = tiles[0][0]
                rev = tiles[0][1] > tiles[-1][1]
                Wc = GRP_WMAX[G]
                # plant slice: fwd groups use pf[:gb]; rev use pr tail
                plant = (pr_sb[:, GB - gb:GB] if rev else pf_sb[:, 0:gb])
                state = {}

                def s_tau():
                    tau32 = taupool.tile([128, gb, 1], mybir.dt.float32,
                                      name=f"tau{G}", tag="tau32")
                    nc.vector.tensor_scalar(tau32, vgrp[:, :, 15:16], 1.0,
                                            0.0, op0=mybir.AluOpType.mult,
                                            op1=mybir.AluOpType.add)
                    cpos = cpool.tile([128, gb, Wc + 1], f16, name=f"cp{G}",
                                     tag="cpos")
                    nc.vector.tensor_scalar(
                        cpos[:, :, 0:1], plant.unsqueeze(2), 1.0,
                        0.0, op0=mybir.AluOpType.mult,
                        op1=mybir.AluOpType.add)
                    state["tau32"] = tau32
                    state["cpos"] = cpos

                def s_cpos(lo, hi, dve=False):
                    eng = nc.vector if dve else nc.gpsimd

                    def f():
                        cpos, tau32 = state["cpos"], state["tau32"]
                        for gl in range(lo, hi):
                            eng.tensor_scalar(
                                cpos[:, gl, 1:], scg[:, gl, :],
                                tau32[:, gl, :], 1.0,
                                op0=mybir.AluOpType.is_ge,
                                op1=mybir.AluOpType.mult)
                    return f

                def s_scan():
                    cpos = state["cpos"]
                    P = ppool.tile([128, gb, Wc + 1], f16, name=f"P{G}",
                                  tag="P")
                    nc.vector.tensor_tensor_scan(
                        P.rearrange("p a b -> p (a b)"),
                        gate_cls[Wc][:, 0:gb, :].rearrange("p a b -> p (a b)"),
                        cpos.rearrange("p a b -> p (a b)"),
                        0.0, op0=mybir.AluOpType.mult,
                        op1=mybir.AluOpType.add)
                    state["P"] = P

                def s_u():
                    u = upool.tile([128, gb, Wc + 1], f16, name=f"u{G}",
                                   tag="u")
                    ueng = nc.gpsimd if (not rev and Wc <= 100) else nc.vector
                    ueng.tensor_scalar(
                        u[:, :, :], state["cpos"][:, :, :], BIGC, 0.0,
                        op0=mybir.AluOpType.mult, op1=mybir.AluOpType.add)
                    state["u"] = u

                def s_idx():
                    idx = ixpool.tile([128, gb * Wc], i16, name=f"ix{G}",
                                    tag="idx")
                    nc.vector.tensor_tensor(
                        idx[:, :].rearrange("p (a b) -> p a b", a=gb),
                        state["u"][:, :, 1:], state["P"][:, :, 1:],
                        mybir.AluOpType.add)
                    state["idx"] = idx

                def s_scatter():
                    dst = dstp.tile([128, gb * BINS], i16, name=f"d{G}",
                                    tag="dst")
                    nc.gpsimd.local_scatter(
                        dst[:, :], iota_cls[Wc][:, 0:gb * Wc],
                        state["idx"][:, :],
                        channels=128, num_elems=gb * BINS, num_idxs=gb * Wc)
                    state["dst"] = dst

                def s_out():
                    dview = state["dst"][:, :].rearrange(
                        "p (a b) -> p a b", a=gb)
                    jb = b * TPB + min(i for _, i in tiles)
                    nc.sync.dma_start(out=out_v[:, jb:jb + gb, :],
                                      in_=dview[:, :, 1:17])

                if G == len(GRPS) - 1:
                    state["cpos"] = lcpos
                    return [s_u, s_scan, s_idx, s_scatter, s_out]
                s_tau()
                if gb == GB:
                    return [s_cpos(0, 7), s_cpos(7, gb, dve=rev), None, None,
                            None, None, None, None, s_u, s_scan, s_idx,
                            s_scatter, s_out]
                h = min(4, gb)
                steps = [s_cpos(0, h)]
                if gb > h:
                    steps.append(s_cpos(h, gb, dve=rev))
                steps += [None, s_u, s_scan, s_idx, s_scatter, s_out]
                return steps

            pending = []
            for G, (s0, gb) in enumerate(GRPS):
                tiles = TILES[s0:s0 + gb]
                Wc = GRP_WMAX[G]
                scg = scgp.tile([128, gb, Wc], f16, name=f"scg{G}", tag="scg")
                vgrp = vpool.tile([128, gb, SEL], f16, name=f"v{G}", tag="v")
                lastg = (G == len(GRPS) - 1)
                if lastg:
                    lcpos = cpool.tile([128, gb, Wc + 1], f16,
                                       name=f"lcp{G}", tag="cpos")
                    nc.vector.tensor_scalar(
                        lcpos[:, :, 0:1], pr_sb[:, GB - gb:GB].unsqueeze(2),
                        1.0, 0.0, op0=mybir.AluOpType.mult,
                        op1=mybir.AluOpType.add)
                for gl, (bb, i) in enumerate(tiles):
                    W = 2 * i + 2
                    colbase = bb * T + i * 128
                    ps = sc_ps.tile([128, Wc], f32, name=f"ps{G}_{gl}",
                                    tag="scps")
                    nc.tensor.matmul(ps, lhsT=qT_sb[:, colbase:colbase + 128],
                                     rhs=M_sb[bb][:, 0:Wc],
                                     start=True, stop=False)
                    nc.tensor.matmul(ps[:, 2 * i + 1:2 * i + 2],
                                     lhsT=zap_sb, rhs=one_sb,
                                     start=False, stop=True)
                    nc.scalar.copy(out=scg[:, gl, :], in_=ps)
                    sc2 = small.tile([128, W], f16, name=f"sc2_{G}_{gl}",
                                     tag="sc2")
                    va = vgrp[:, gl, 0:8]
                    vb = vgrp[:, gl, 8:16]
                    nc.vector.max(out=va, in_=scg[:, gl, 0:W])
                    nc.vector.match_replace(out=sc2, in_to_replace=va,
                                            in_values=scg[:, gl, 0:W],
                                            imm_value=ZAPV)
                    nc.vector.max(out=vb, in_=sc2)
                    if lastg:
                        ltau = taupool.tile([128, 1], mybir.dt.float32,
                                            name=f"lt{G}_{gl}", tag="ltau")
                        nc.vector.tensor_scalar(
                            ltau, vgrp[:, gl, 15:16], 1.0, 0.0,
                            op0=mybir.AluOpType.mult,
                            op1=mybir.AluOpType.add)
                        nc.vector.tensor_scalar(
                            lcpos[:, gl, 1:], scg[:, gl, :], ltau, 1.0,
                            op0=mybir.AluOpType.is_ge,
                            op1=mybir.AluOpType.mult)
                    if pending:
                        s = pending.pop(0)
                        if s is not None:
                            s()
                for s in pending:       # drain any leftovers at group end
                    if s is not None:
                        s()
                if G == 1:
                    make_m(1)
                    emit_early_out()
                pending = extraction_steps(G, scg, vgrp)
            for s in pending:
                if s is not None:
                    s()

    nc.compile()
    return nc


def _shard_inputs(Q, K, Wq, Wk):
    early, pf, pr, zap, one, blkind = _static_tables()
    in_maps = []
    for h in range(H):
        qT = np.ascontiguousarray(
            Q[:, :, GROUPS * h, :].reshape(ROWS, D).T).astype(np.float16)
        kcs = {}
        for b in range(B):
            kb = K[b, :, h, :].reshape(64, 128, D).transpose(1, 0, 2)
            kcs[f"kc{b}"] = np.ascontiguousarray(
                kb.reshape(128, 64 * D)).astype(np.float16)
        G = (Wq[h].astype(np.float64)
             @ Wk[h].astype(np.float64).T / BS).astype(np.float32)
        bun16 = np.hstack([blkind, pf, pr]).astype(np.float16)
        zapone = np.hstack([zap, one]).astype(np.float16)
        buni = early.astype(np.int16)
        in_maps.append({
            "qT": qT, **kcs,
            "gT": np.ascontiguousarray(G.T),
            "bun16": bun16, "zapone": zapone, "buni": buni,
        })
    return in_maps


def kernel(Q, K, Wq, Wk, logit_scale=None, block_size=64, selected_blocks=16,
           groups=4, **_unused):
    assert int(block_size) == BS and int(selected_blocks) == SEL
    assert int(groups) == GROUPS
    Q = np.asarray(Q, np.float32)
    K = np.asarray(K, np.float32)
    Wq = np.asarray(Wq, np.float32)
    Wk = np.asarray(Wk, np.float32)
    # exp(logit_scale) > 0 scales scores per-head only -> ranking unchanged.

    if "nc" not in _CACHE:
        _CACHE["nc"] = build_program()
    nc = _CACHE["nc"]

    in_maps = _shard_inputs(Q, K, Wq, Wk)
    res = run_bass_kernel_spmd(nc, in_maps, core_ids=list(range(H)))
    outs = [res.results[h]["out"] for h in range(H)]          # [ROWS, SEL] i16
    out = np.stack(outs, axis=1).reshape(B, T, H, SEL).astype(np.int32)
    # union-with-locals clamp: out[..., 15] = min(out[..., 15], t_blk - 1)
    # (early rows t < 1024 come from the static table and are left as-is)
    tbm1 = np.maximum(np.arange(T) // BS - 1, 0).astype(np.int32)
    out[:, 1024:, :, 15] = np.minimum(out[:, 1024:, :, 15],
                                      tbm1[1024:, None])
    return out


if __name__ == "__main__":
    rng = np.random.default_rng(0)
    Q = rng.standard_normal((B, T, HQ, D)).astype(np.float32)
    K = rng.standard_normal((B, T, H, D)).astype(np.float32)
    Wq = (rng.standard_normal((H, D, DR)) * 0.02).astype(np.float32)
    Wk = (rng.standard_normal((H, D, DR)) * 0.02).astype(np.float32)
    out = kernel(Q=Q, K=K, Wq=Wq, Wk=Wk)
    print("kernel ran:", out.shape, out.dtype)
